# revision 15
# baseline (speedup 1.0000x reference)
"""Multi-head attention (B=2, S=2048, E=1024, H=16) on 8 TRN2 NeuronCores.

Sharding: batch x head-group. Core c handles batch c//4 and heads
(c%4)*4 .. +3, as 2 groups x 2 head-pairs. Pair A lives in SBUF/array
partitions 0-63, pair B in 64-127, enabling 2x row/col-tiled matmuls
(64-row PE tiles T0/T8) for the QK product and all projections.

Per (qb, kc4) block of the attention matrix, one of four elementwise
plans computes pt = exp(sim*mask):
  A: DVE mul (psum x mask -> bf16) + ACT true exp (in-place sbuf)
  B: DVE mul (mask pre-scaled by A16) + DVE tensor_scalar add B16 ->
     int16 bits == bf16(exp) (Schraudolph fast-exp; max ~4e-3 final err)
  G: DVE mul + GpSimd tensor_scalar (fast-exp on Q7)
  C: ACT copy psum->sbuf + GpSimd mul + DVE tensor_scalar (fast-exp)
The mix balances DVE/ACT/GpSimd occupancy; the mask tensor is
region-scaled host-side (x A16 for fast-exp regions).

PV keeps the ones-column trick ([V | 1] stationary, 65-col output whose
row 64 is the softmax denominator) and is software-pipelined one q-block
behind QK so the PE never stalls on the elementwise pipeline. The Wo
contraction for q-span qb is emitted inside group 1's attention loop as
soon as both groups' outT spans are ready, hiding the output-projection
tail. Reciprocal via DRAM-bounce repack as before."""
import sys

if "/opt/trn_rl_repo" not in sys.path:
    sys.path.insert(0, "/opt/trn_rl_repo")

from contextlib import ExitStack

import ml_dtypes
import numpy as np

B, S, E = 2, 2048, 1024
H = 16
HD = 64
KD = 64
VD = 64
SCALE = 1.0 / np.float32(np.sqrt(np.float32(KD)))
N_CORES = 8
HPC = H // 4  # heads per core = 4
QB = 512  # q-block width
NKC = S // 128  # 16 k-chunks
A16 = np.float64(128.0 / np.log(2.0))  # Schraudolph bf16 scale
B16 = np.float64(127.0 * 128.0 - 7.4)  # Schraudolph bf16 offset

# plan per (qb, kc4) cell; kc4 = kc2//2 indexes [128, 2048] spans.
# 'A' true exp; 'B' DVE fast-exp; 'G' GpSimd fast-exp; 'C' GpSimd mul.
PLAN = {
    (0, 0): "A", (0, 1): "C", (0, 2): "B", (0, 3): "C",
    (1, 0): "A", (1, 1): "C", (1, 2): "A", (1, 3): "C",
    (2, 0): "A", (2, 1): "C", (2, 2): "B", (2, 3): "C",
    (3, 0): "A", (3, 1): "C", (3, 2): "A", (3, 3): "C",
}

_RUNTIME = {}


def _build_nc(repeat=1):
    import concourse.bass as bass
    import concourse.tile as tile
    from concourse import mybir, bacc

    F32 = mybir.dt.float32
    F32R = mybir.dt.float32r
    BF16 = mybir.dt.bfloat16
    I16 = mybir.dt.int16
    Copy = mybir.ActivationFunctionType.Copy
    Ident = mybir.ActivationFunctionType.Identity
    Exp = mybir.ActivationFunctionType.Exp
    Add = mybir.AluOpType.add

    nc = bacc.Bacc("TRN2")
    xt_d = nc.dram_tensor("xt", (4 * HD, S), F32, kind="ExternalInput")
    mask_d = nc.dram_tensor("maskt", (128, NKC * S), BF16, kind="ExternalInput")
    wp_d = nc.dram_tensor("wpack", (128, 3 * 64 + 2), F32, kind="ExternalInput")
    wqkb_d = nc.dram_tensor("wqkb", (128, 128), BF16, kind="ExternalInput")
    xtb_d = nc.dram_tensor("xtb", (4 * HD, S), BF16, kind="ExternalInput")
    wo_d = nc.dram_tensor("wo", (4 * VD, E), BF16, kind="ExternalInput")
    out_d = nc.dram_tensor("partial", (S, E), BF16, kind="ExternalOutput")

    with tile.TileContext(nc) as tc:
        with ExitStack() as ctx:
            const = ctx.enter_context(tc.tile_pool(name="const", bufs=1))
            qkp = ctx.enter_context(tc.tile_pool(name="qkp", bufs=2))
            vtp = ctx.enter_context(tc.tile_pool(name="vtp", bufs=2))
            ptp = ctx.enter_context(tc.tile_pool(name="ptp", bufs=2))
            ctp = ctx.enter_context(tc.tile_pool(name="ctp", bufs=2))
            otbp = ctx.enter_context(tc.tile_pool(name="otbp", bufs=2))
            ot2p = ctx.enter_context(tc.tile_pool(name="ot2p", bufs=2))
            denp = ctx.enter_context(tc.tile_pool(name="denp", bufs=2))
            wst = ctx.enter_context(tc.tile_pool(name="wst", bufs=2))
            drp = ctx.enter_context(tc.tile_pool(name="drp", bufs=2, space="DRAM"))
            simp = ctx.enter_context(tc.tile_pool(name="simp", bufs=2, space="PSUM"))
            accp = ctx.enter_context(tc.tile_pool(name="accp", bufs=4, space="PSUM"))

            # ---- constant loads ----
            wp_sb = const.tile([128, 3 * 64 + 2], F32, tag="wp")
            nc.sync.dma_start(out=wp_sb.bitcast(F32R), in_=wp_d[:, :].bitcast(F32R))
            wq_sb = wp_sb[:, 0:64]
            wk_sb = wp_sb[:, 64:128]
            wv_sb = wp_sb[:, 128:192]
            bq_sb = wp_sb[:, 192:193]
            bk_sb = wp_sb[:, 193:194]
            mask_sb = const.tile([128, NKC * S], BF16, tag="mask")
            for kc in range(NKC):
                eng = nc.scalar if kc % 2 == 0 else nc.sync
                eng.dma_start(
                    out=mask_sb[:, kc * S : (kc + 1) * S],
                    in_=mask_d[:, kc * S : (kc + 1) * S],
                )
            wqkb_sb = const.tile([128, 128], BF16, tag="wqkb")
            nc.sync.dma_start(out=wqkb_sb, in_=wqkb_d[:, :])
            wo_sb = []
            for g in range(2):
                t = const.tile([128, E], BF16, tag=f"wo{g}")
                nc.scalar.dma_start(out=t, in_=wo_d[g * 128 : (g + 1) * 128, :])
                wo_sb.append(t)

            def emit_wo(qc):
                ost = wst.tile([128, E], BF16, tag="wst", name=f"ost_{qc}")
                for e2 in range(2):
                    wo_ps = simp.tile([128, 512], F32, tag="sim", name=f"wops_{qc}_{e2}")
                    for gi in range(2):
                        nc.tensor.matmul(
                            wo_ps[:, :],
                            ot2s[gi][:, qc * 128 : (qc + 1) * 128],
                            wo_sb[gi][:, e2 * 512 : (e2 + 1) * 512],
                            start=(gi == 0),
                            stop=(gi == 1),
                        )
                    dst = ost[:, e2 * 512 : (e2 + 1) * 512]
                    if (qc + e2) % 2 == 0:
                        nc.scalar.activation(dst, wo_ps[:, :], Copy)
                    else:
                        nc.vector.tensor_copy(dst, wo_ps[:, :])
                eng = nc.sync if qc % 2 == 0 else nc.scalar
                eng.dma_start(out=out_d[qc * 128 : (qc + 1) * 128, :], in_=ost)

            for rep in range(repeat):
                ot2s = []
                for g in range(2):
                    xt_g = const.tile([128, S], F32, tag="xt", name=f"xt_g{g}_r{rep}")
                    nc.sync.dma_start(
                        out=xt_g.bitcast(F32R),
                        in_=xt_d[g * 128 : (g + 1) * 128, :].bitcast(F32R),
                    )
                    xtb_g = const.tile([128, S], BF16, tag="xtb", name=f"xtb_g{g}_r{rep}")
                    nc.scalar.dma_start(
                        out=xtb_g, in_=xtb_d[g * 128 : (g + 1) * 128, :]
                    )
                    # ---- Q/K projections: row+col tiled (T0 / T10) ----
                    qt = qkp.tile([128, S], BF16, tag="qt", name=f"qt_g{g}_r{rep}")
                    kt = qkp.tile([128, S], BF16, tag="kt", name=f"kt_g{g}_r{rep}")
                    for sp in range(S // 512):
                        ssl = slice(sp * 512, (sp + 1) * 512)
                        for dst, wcol, b_sb in ((qt, 0, bq_sb), (kt, 64, bk_sb)):
                            ps = simp.tile([128, 512], F32, tag="sim")
                            nc.tensor.matmul(
                                ps[0:64, :],
                                wqkb_sb[0:64, wcol : wcol + 64],
                                xtb_g[0:64, ssl],
                                start=True, stop=True,
                                tile_position=(0, 0),
                            )
                            nc.tensor.matmul(
                                ps[64:128, :],
                                wqkb_sb[64:128, wcol : wcol + 64],
                                xtb_g[64:128, ssl],
                                start=True, stop=True,
                                tile_position=(64, 64),
                            )
                            nc.scalar.activation(
                                dst[:, ssl], ps[:, :], Ident, bias=b_sb[0:128, :]
                            )

                    # ---- V projections: row tiled, [V | 1] layout ----
                    vts = []
                    for p01 in range(2):
                        vt = vtp.tile(
                            [128, NKC * 65], BF16, tag=f"vt{p01}",
                            name=f"v_g{g}_{p01}_r{rep}",
                        )
                        ones_ap = vt.rearrange("p (c k) -> p c k", k=65)[:, :, 64:65]
                        nc.gpsimd.memset(ones_ap, 1.0)
                        vts.append(vt)
                    for vc4 in range(NKC // 4):
                        vps = [accp.tile([128, 256], F32, tag="acc", name=f"vps{p}_g{g}_c{vc4}_r{rep}") for p in range(2)]
                        for j in range(4):
                            sc = vc4 * 4 + j
                            for p01 in range(2):
                                rsl = slice(p01 * 64, p01 * 64 + 64)
                                nc.tensor.matmul(
                                    vps[p01][:, j * 64 : (j + 1) * 64],
                                    xt_g[rsl, sc * 128 : (sc + 1) * 128].bitcast(F32R),
                                    wv_sb[rsl, :].bitcast(F32R),
                                    start=True, stop=True,
                                    tile_position=(p01 * 64, 0),
                                )
                        for p01 in range(2):
                            dst = vts[p01].rearrange("p (c k) -> p c k", k=65)[
                                :, vc4 * 4 : (vc4 + 1) * 4, 0:64
                            ]
                            src = vps[p01].rearrange("p (c k) -> p c k", k=64)
                            if p01 == 0:
                                nc.scalar.activation(dst, src, Copy)
                            else:
                                nc.vector.tensor_copy(dst, src)

                    # ---- attention: QK row-tiled; PV pipelined one qb behind ----
                    ot2 = ot2p.tile([128, S], BF16, tag="ot2", name=f"ot2_g{g}_r{rep}")
                    ot2s.append(ot2)
                    prev = None         # (pts, qb) awaiting PV
                    out_pending = None  # (recbs, pvs, qb) awaiting out-mul
                    wo_pending = []     # qc indices ready for output projection

                    def emit_chain(pvs):
                        """Start denominator -> reciprocal -> broadcast chain."""
                        recbs = []
                        for p01 in range(2):
                            den = denp.tile([1, QB], F32, tag="den")
                            nc.scalar.activation(den, pvs[p01][64:65, :], Copy)
                            dden = drp.tile([1, QB], F32, tag="dden")
                            nc.sync.dma_start(out=dden, in_=den)
                            dpk = denp.tile([128, QB // 128], F32, tag="dpk")
                            nc.sync.dma_start(
                                out=dpk,
                                in_=dden.rearrange("a (p f) -> (a p) f", p=128),
                            )
                            rpk = denp.tile([128, QB // 128], BF16, tag="rpk")
                            with nc.allow_low_precision(reason="bf16 1/den: ~0.2% softmax scale error, within tolerance"):
                                nc.vector.reciprocal(rpk, dpk)
                            drec = drp.tile([1, QB], BF16, tag="drec")
                            nc.sync.dma_start(
                                out=drec.rearrange("a (p f) -> (a p) f", p=128),
                                in_=rpk,
                            )
                            recb = denp.tile([64, QB], BF16, tag="recb")
                            nc.sync.dma_start(
                                out=recb,
                                in_=bass.AP(
                                    tensor=drec.tensor,
                                    offset=drec.offset,
                                    ap=[[0, 64]] + [list(a) for a in drec.ap[1:]],
                                ),
                            )
                            recbs.append(recb)
                        return recbs

                    def emit_outmul(recbs, pvs, fqb):
                        fqsl = slice(fqb * QB, (fqb + 1) * QB)
                        # pair A: lane-aligned direct write into ot2 rows 0-63
                        nc.vector.tensor_mul(
                            ot2[0:64, fqsl], pvs[0][0:64, :], recbs[0]
                        )
                        # pair B: psum rows 0-63 -> ot2 rows 64-127 via DMA shift
                        otb = otbp.tile([64, QB], BF16, tag="otb")
                        nc.vector.tensor_mul(otb, pvs[1][0:64, :], recbs[1])
                        nc.scalar.dma_start(out=ot2[64:128, fqsl], in_=otb)

                    for qb in range(S // QB):
                        qsl = slice(qb * QB, (qb + 1) * QB)
                        pts = [
                            ptp.tile([128, NKC * QB], BF16, tag=f"pt{p}",
                                     name=f"pt{p}_g{g}_q{qb}_r{rep}")
                            for p in range(2)
                        ]
                        if prev is not None:
                            pvs_prev = [
                                accp.tile([65, QB], F32, tag="acc",
                                          name=f"pv{p}_g{g}_q{qb - 1}_r{rep}")
                                for p in range(2)
                            ]
                        ctmps = [None, None]
                        for kc in range(NKC):
                            kc2 = kc // 2
                            j2 = kc % 2
                            plan = PLAN[(qb, kc2 // 2)]
                            sims = [simp.tile([128, 512], F32, tag="sim", name=f"sim{p}_g{g}_q{qb}_k{kc}_r{rep}") for p in range(2)]
                            for p01 in range(2):
                                rsl = slice(p01 * 64, p01 * 64 + 64)
                                nc.tensor.matmul(
                                    sims[p01][:, :],
                                    kt[rsl, kc * 128 : (kc + 1) * 128],
                                    qt[rsl, qsl],
                                    start=True, stop=True,
                                    tile_position=(p01 * 64, 0),
                                )
                            if prev is not None:
                                ptsp = prev[0]
                                for p01 in range(2):
                                    nc.tensor.matmul(
                                        pvs_prev[p01][:, :],
                                        vts[p01][:, kc * 65 : (kc + 1) * 65],
                                        ptsp[p01][:, kc * QB : (kc + 1) * QB],
                                        start=(kc == 0),
                                        stop=(kc == NKC - 1),
                                    )
                            moff = (kc2 * (S // QB) + qb) * 1024 + j2 * 512
                            span = slice(kc * QB, (kc + 1) * QB)
                            for p01 in range(2):
                                if plan == "C":
                                    if j2 == 0:
                                        ctmps[p01] = ctp.tile(
                                            [128, 1024], BF16, tag=f"ct{p01}",
                                            name=f"ct{p01}_g{g}_q{qb}_k{kc}_r{rep}",
                                        )
                                    nc.scalar.activation(
                                        ctmps[p01][:, j2 * 512 : (j2 + 1) * 512],
                                        sims[p01][:, :], Copy,
                                    )
                                    if j2 == 1:
                                        moff2 = (kc2 * (S // QB) + qb) * 1024
                                        span2k = slice((kc - 1) * QB, (kc + 1) * QB)
                                        nc.gpsimd.tensor_mul(
                                            pts[p01][:, span2k], ctmps[p01],
                                            mask_sb[:, moff2 : moff2 + 1024],
                                        )
                                else:
                                    nc.vector.tensor_mul(
                                        pts[p01][:, span], sims[p01][:, :],
                                        mask_sb[:, moff : moff + 512],
                                    )
                            if kc % 4 == 3:
                                span2 = slice((kc - 3) * QB, (kc + 1) * QB)
                                for p01 in range(2):
                                    seg = pts[p01][:, span2]
                                    if plan == "A":
                                        nc.scalar.activation(seg, seg, Exp)
                                    else:  # B, C -> DVE fast-exp add
                                        nc.vector.tensor_scalar(
                                            seg.bitcast(I16), seg, float(B16), None,
                                            op0=Add,
                                        )
                            if kc == 5 and out_pending is not None:
                                emit_outmul(*out_pending)
                                wo_qb = out_pending[2]
                                out_pending = None
                                if g == 1:
                                    wo_pending = list(range(4 * wo_qb, 4 * wo_qb + 4))
                            if kc in (8, 10, 12, 14) and wo_pending:
                                emit_wo(wo_pending.pop(0))
                        while wo_pending:
                            emit_wo(wo_pending.pop(0))
                        if prev is not None:
                            out_pending = (emit_chain(pvs_prev), pvs_prev, prev[1])
                        prev = (pts, qb)

                    # tail: PV + finish for the last q-block of this group
                    pvs_last = [
                        accp.tile([65, QB], F32, tag="acc",
                                  name=f"pv{p}_g{g}_q3_r{rep}")
                        for p in range(2)
                    ]
                    for c in range(NKC):
                        for p01 in range(2):
                            nc.tensor.matmul(
                                pvs_last[p01][:, :],
                                vts[p01][:, c * 65 : (c + 1) * 65],
                                prev[0][p01][:, c * QB : (c + 1) * QB],
                                start=(c == 0),
                                stop=(c == NKC - 1),
                            )
                    if out_pending is not None:
                        emit_outmul(*out_pending)
                        if g == 1:
                            wo_pending = list(range(8, 12))
                    ch_last = emit_chain(pvs_last)
                    if g == 1:
                        while wo_pending:
                            emit_wo(wo_pending.pop(0))
                    emit_outmul(ch_last, pvs_last, 3)

                # ---- output projection (remaining spans) ----
                for qc in range(12, 16):
                    emit_wo(qc)
    nc.finalize()
    return nc


def _build_runner(repeat=1):
    """Compile once. Returns an object with prep/exec/reduce/run (see use
    in kernel() and test.py)."""
    import jax
    import jax.numpy as jnp
    import numpy as _np
    from jax.experimental.shard_map import shard_map
    from jax.sharding import Mesh, NamedSharding, PartitionSpec

    from concourse import mybir
    from concourse.bass2jax import (
        _bass_exec_p,
        install_neuronx_cc_hook,
        partition_id_tensor,
    )

    nc = _build_nc(repeat=repeat)
    install_neuronx_cc_hook()
    partition_name = nc.partition_id_tensor.name if nc.partition_id_tensor else None

    replicated = {"maskt", "wpack", "wqkb"}

    in_names, out_names, out_avals, out_shapes, out_dtypes = [], [], [], [], []
    for alloc in nc.m.functions[0].allocations:
        if not isinstance(alloc, mybir.MemoryLocationSet):
            continue
        name = alloc.memorylocations[0].name
        if alloc.kind == "ExternalInput":
            if name != partition_name:
                in_names.append(name)
        elif alloc.kind == "ExternalOutput":
            out_names.append(name)
            shape = tuple(alloc.tensor_shape)
            dtype = mybir.dt.np(alloc.dtype)
            out_avals.append(jax.core.ShapedArray(shape, dtype))
            out_shapes.append(shape)
            out_dtypes.append(dtype)

    n_params = len(in_names)
    n_outs = len(out_names)
    all_in_names = list(in_names) + list(out_names)
    if partition_name is not None:
        all_in_names.append(partition_name)
    donate = tuple(range(n_params, n_params + n_outs))

    def _body(*args):
        operands = list(args)
        if partition_name is not None:
            operands.append(partition_id_tensor())
        outs = _bass_exec_p.bind(
            *operands,
            out_avals=tuple(out_avals),
            in_names=tuple(all_in_names),
            out_names=tuple(out_names),
            lowering_input_output_aliases=(),
            sim_require_finite=True,
            sim_require_nnan=True,
            nc=nc,
        )
        return tuple(outs)

    devices = jax.devices()[:N_CORES]
    mesh = Mesh(_np.asarray(devices), ("core",))
    shard0 = NamedSharding(mesh, PartitionSpec("core"))
    srepl = NamedSharding(mesh, PartitionSpec())
    in_specs = tuple(
        PartitionSpec() if name in replicated else PartitionSpec("core")
        for name in in_names
    ) + (PartitionSpec("core"),) * n_outs
    out_specs = (PartitionSpec("core"),) * n_outs

    sharded = jax.jit(
        shard_map(
            _body, mesh=mesh, in_specs=in_specs, out_specs=out_specs,
            check_rep=False,
        ),
        donate_argnums=donate,
        keep_unused=True,
    )

    _zeros = jax.jit(
        lambda: tuple(
            jnp.zeros((N_CORES * s[0], *s[1:]), d)
            for s, d in zip(out_shapes, out_dtypes)
        ),
        out_shardings=(shard0,) * n_outs,
    )

    _reduce = jax.jit(
        lambda p: p.reshape(B, 4, S, E).sum(axis=1).reshape(B * S, E),
        out_shardings=shard0,
    )

    def prep(in_maps):
        args = []
        for name in in_names:
            if name in replicated:
                arr = _np.asarray(in_maps[0][name])
                args.append(jax.device_put(arr, srepl))
            else:
                arr = _np.concatenate(
                    [_np.asarray(m[name]) for m in in_maps], axis=0
                )
                args.append(jax.device_put(arr, shard0))
        return args

    def make_zeros():
        return _zeros()

    def exec_device(args, zeros=None):
        if zeros is None:
            zeros = _zeros()
        outs = sharded(*args, *zeros)
        return jax.block_until_ready(outs[0])

    def exec_async(args, zeros):
        return sharded(*args, *zeros)[0]

    def reduce_device(partials):
        return jax.block_until_ready(_reduce(partials))

    def run(in_maps):
        partials = exec_device(prep(in_maps))
        return _np.asarray(reduce_device(partials))  # (B*S, E)

    class R:
        pass

    r = R()
    r.nc = nc
    r.prep = prep
    r.make_zeros = make_zeros
    r.exec_device = exec_device
    r.exec_async = exec_async
    r.reduce_device = reduce_device
    r.run = run
    return r


def _runtime(repeat=1):
    if repeat not in _RUNTIME:
        _RUNTIME[repeat] = _build_runner(repeat=repeat)
    return _RUNTIME[repeat]


def make_in_maps(x, mask, Wq, bq, Wk, bk, Wv, bv, Wo, bo):
    bf16 = ml_dtypes.bfloat16
    x = np.asarray(x, np.float32)
    m = np.asarray(mask, np.float32).T  # [k, q]
    # device layout: [128, (kc2, qb, j, ql)]; each mul reads one flat
    # [128, 1024] span at moff=(kc2*4+qb)*1024
    maskT = np.ascontiguousarray(
        m.reshape(NKC // 2, 2, 128, S // QB, QB)
        .transpose(2, 0, 3, 1, 4)
        .reshape(128, NKC * S)
    ).astype(np.float32)
    # scale fast-exp regions by A16 (Schraudolph): all plans except 'A'
    mview = maskT.reshape(128, NKC // 2, S // QB, 2 * QB)
    for kc2 in range(NKC // 2):
        for qb in range(S // QB):
            if PLAN[(qb, kc2 // 2)] != "A":
                mview[:, kc2, qb, :] *= np.float32(A16)
    maskT = maskT.astype(bf16)

    wq_s = (np.asarray(Wq, np.float32) * SCALE).astype(np.float32)
    bq_s = (np.asarray(bq, np.float32) * SCALE).astype(np.float32)
    wq2 = np.concatenate([wq_s, wq_s], axis=0)
    wk2 = np.concatenate([np.asarray(Wk, np.float32)] * 2, axis=0)
    wv2 = np.concatenate([np.asarray(Wv, np.float32)] * 2, axis=0)
    bq2 = np.concatenate([bq_s, bq_s])[:, None].astype(np.float32)
    bk2 = np.concatenate([np.asarray(bk, np.float32)] * 2)[:, None].astype(np.float32)
    wpack = np.ascontiguousarray(
        np.concatenate([wq2, wk2, wv2, bq2, bk2], axis=1), np.float32
    )
    wqkb = np.ascontiguousarray(
        np.concatenate([wq2, wk2], axis=1)
    ).astype(bf16)

    in_maps = []
    for c in range(N_CORES):
        b = c // 4
        h0 = (c % 4) * HPC
        r0 = h0 * HD
        xt = np.ascontiguousarray(x[b].T[r0 : r0 + HPC * HD, :])
        xtb = xt.astype(bf16)
        wo = np.ascontiguousarray(np.asarray(Wo, np.float32)[r0 : r0 + HPC * VD, :]).astype(bf16)
        in_maps.append(
            {
                "xt": xt,
                "xtb": xtb,
                "maskt": maskT,
                "wpack": wpack,
                "wqkb": wqkb,
                "wo": wo,
            }
        )
    return in_maps


def kernel(x, mask, Wq, bq, Wk, bk, Wv, bv, Wo, bo):
    r = _runtime()
    in_maps = make_in_maps(x, mask, Wq, bq, Wk, bk, Wv, bv, Wo, bo)
    flat = r.run(in_maps)  # (B*S, E), per-batch partials already summed
    Wo32 = np.asarray(Wo, np.float32)
    crow = np.asarray(bo, np.float32) + np.tile(np.asarray(bv, np.float32), H) @ Wo32
    out = flat.reshape(B, S, E) + crow[None, None, :]
    return out.astype(np.float32)


# revision 16
# speedup vs baseline: 1.0006x; 1.0006x over previous
"""Multi-head attention (B=2, S=2048, E=1024, H=16) on 8 TRN2 NeuronCores.

Sharding: batch x head-group. Core c handles batch c//4 and heads
(c%4)*4 .. +3, as 2 groups x 2 head-pairs. Pair A lives in SBUF/array
partitions 0-63, pair B in 64-127, enabling 2x row/col-tiled matmuls
(64-row PE tiles T0/T8) for the QK product and all projections.

Per (qb, kc4) block of the attention matrix, one of four elementwise
plans computes pt = exp(sim*mask):
  A: DVE mul (psum x mask -> bf16) + ACT true exp (in-place sbuf)
  B: DVE mul (mask pre-scaled by A16) + DVE tensor_scalar add B16 ->
     int16 bits == bf16(exp) (Schraudolph fast-exp; max ~4e-3 final err)
  G: DVE mul + GpSimd tensor_scalar (fast-exp on Q7)
  C: ACT copy psum->sbuf + GpSimd mul + DVE tensor_scalar (fast-exp)
The mix balances DVE/ACT/GpSimd occupancy; the mask tensor is
region-scaled host-side (x A16 for fast-exp regions).

PV keeps the ones-column trick ([V | 1] stationary, 65-col output whose
row 64 is the softmax denominator) and is software-pipelined one q-block
behind QK so the PE never stalls on the elementwise pipeline. The Wo
contraction for q-span qb is emitted inside group 1's attention loop as
soon as both groups' outT spans are ready, hiding the output-projection
tail. Reciprocal via DRAM-bounce repack as before."""
import sys

if "/opt/trn_rl_repo" not in sys.path:
    sys.path.insert(0, "/opt/trn_rl_repo")

from contextlib import ExitStack

import ml_dtypes
import numpy as np

B, S, E = 2, 2048, 1024
H = 16
HD = 64
KD = 64
VD = 64
SCALE = 1.0 / np.float32(np.sqrt(np.float32(KD)))
N_CORES = 8
HPC = H // 4  # heads per core = 4
QB = 512  # q-block width
NKC = S // 128  # 16 k-chunks
A16 = np.float64(128.0 / np.log(2.0))  # Schraudolph bf16 scale
B16 = np.float64(127.0 * 128.0 - 7.4)  # Schraudolph bf16 offset

# plan per (qb, kc4) cell; kc4 = kc2//2 indexes [128, 2048] spans.
# 'A' true exp; 'B' DVE fast-exp; 'G' GpSimd fast-exp; 'C' GpSimd mul.
PLAN = {
    (0, 0): "A", (0, 1): "C", (0, 2): "B", (0, 3): "C",
    (1, 0): "A", (1, 1): "C", (1, 2): "A", (1, 3): "C",
    (2, 0): "A", (2, 1): "C", (2, 2): "B", (2, 3): "C",
    (3, 0): "A", (3, 1): "C", (3, 2): "A", (3, 3): "C",
}

_RUNTIME = {}


def _build_nc(repeat=1):
    import concourse.bass as bass
    import concourse.tile as tile
    from concourse import mybir, bacc

    F32 = mybir.dt.float32
    F32R = mybir.dt.float32r
    BF16 = mybir.dt.bfloat16
    I16 = mybir.dt.int16
    Copy = mybir.ActivationFunctionType.Copy
    Ident = mybir.ActivationFunctionType.Identity
    Exp = mybir.ActivationFunctionType.Exp
    Add = mybir.AluOpType.add

    nc = bacc.Bacc("TRN2")
    xt_d = nc.dram_tensor("xt", (4 * HD, S), F32, kind="ExternalInput")
    mask_d = nc.dram_tensor("maskt", (128, NKC * S), BF16, kind="ExternalInput")
    wp_d = nc.dram_tensor("wpack", (128, 3 * 64 + 2), F32, kind="ExternalInput")
    wqkb_d = nc.dram_tensor("wqkb", (128, 128), BF16, kind="ExternalInput")
    xtb_d = nc.dram_tensor("xtb", (4 * HD, S), BF16, kind="ExternalInput")
    wo_d = nc.dram_tensor("wo", (4 * VD, E), BF16, kind="ExternalInput")
    out_d = nc.dram_tensor("partial", (S, E), BF16, kind="ExternalOutput")

    with tile.TileContext(nc) as tc:
        with ExitStack() as ctx:
            const = ctx.enter_context(tc.tile_pool(name="const", bufs=1))
            qkp = ctx.enter_context(tc.tile_pool(name="qkp", bufs=2))
            vtp = ctx.enter_context(tc.tile_pool(name="vtp", bufs=2))
            ptp = ctx.enter_context(tc.tile_pool(name="ptp", bufs=2))
            ctp = ctx.enter_context(tc.tile_pool(name="ctp", bufs=2))
            otbp = ctx.enter_context(tc.tile_pool(name="otbp", bufs=2))
            ot2p = ctx.enter_context(tc.tile_pool(name="ot2p", bufs=2))
            denp = ctx.enter_context(tc.tile_pool(name="denp", bufs=2))
            wst = ctx.enter_context(tc.tile_pool(name="wst", bufs=2))
            drp = ctx.enter_context(tc.tile_pool(name="drp", bufs=2, space="DRAM"))
            simp = ctx.enter_context(tc.tile_pool(name="simp", bufs=2, space="PSUM"))
            accp = ctx.enter_context(tc.tile_pool(name="accp", bufs=4, space="PSUM"))

            # ---- constant loads ----
            wp_sb = const.tile([128, 3 * 64 + 2], F32, tag="wp")
            nc.sync.dma_start(out=wp_sb.bitcast(F32R), in_=wp_d[:, :].bitcast(F32R))
            wq_sb = wp_sb[:, 0:64]
            wk_sb = wp_sb[:, 64:128]
            wv_sb = wp_sb[:, 128:192]
            bq_sb = wp_sb[:, 192:193]
            bk_sb = wp_sb[:, 193:194]
            mask_sb = const.tile([128, NKC * S], BF16, tag="mask")
            for kc in range(NKC):
                eng = nc.scalar if kc % 2 == 0 else nc.sync
                eng.dma_start(
                    out=mask_sb[:, kc * S : (kc + 1) * S],
                    in_=mask_d[:, kc * S : (kc + 1) * S],
                )
            wqkb_sb = const.tile([128, 128], BF16, tag="wqkb")
            nc.sync.dma_start(out=wqkb_sb, in_=wqkb_d[:, :])
            wo_sb = []
            for g in range(2):
                t = const.tile([128, E], BF16, tag=f"wo{g}")
                nc.scalar.dma_start(out=t, in_=wo_d[g * 128 : (g + 1) * 128, :])
                wo_sb.append(t)

            def emit_wo(qc):
                ost = wst.tile([128, E], BF16, tag="wst", name=f"ost_{qc}")
                for e2 in range(2):
                    wo_ps = simp.tile([128, 512], F32, tag="sim", name=f"wops_{qc}_{e2}")
                    for gi in range(2):
                        nc.tensor.matmul(
                            wo_ps[:, :],
                            ot2s[gi][:, qc * 128 : (qc + 1) * 128],
                            wo_sb[gi][:, e2 * 512 : (e2 + 1) * 512],
                            start=(gi == 0),
                            stop=(gi == 1),
                        )
                    dst = ost[:, e2 * 512 : (e2 + 1) * 512]
                    if (qc + e2) % 2 == 0:
                        nc.scalar.activation(dst, wo_ps[:, :], Copy)
                    else:
                        nc.vector.tensor_copy(dst, wo_ps[:, :])
                eng = nc.sync if qc % 2 == 0 else nc.scalar
                eng.dma_start(out=out_d[qc * 128 : (qc + 1) * 128, :], in_=ost)

            for rep in range(repeat):
                ot2s = []
                # pipeline state carried across group boundaries
                prev = None         # dict(pts, vts, ot2, qb) awaiting PV
                out_pending = None  # (recbs, pvs, qb, g, ot2) awaiting out-mul
                wo_pending = []     # qc indices ready for output projection

                def emit_chain(pvs):
                    """Start denominator -> reciprocal -> broadcast chain."""
                    recbs = []
                    for p01 in range(2):
                        den = denp.tile([1, QB], F32, tag="den")
                        nc.scalar.activation(den, pvs[p01][64:65, :], Copy)
                        dden = drp.tile([1, QB], F32, tag="dden")
                        nc.sync.dma_start(out=dden, in_=den)
                        dpk = denp.tile([128, QB // 128], F32, tag="dpk")
                        nc.sync.dma_start(
                            out=dpk,
                            in_=dden.rearrange("a (p f) -> (a p) f", p=128),
                        )
                        rpk = denp.tile([128, QB // 128], BF16, tag="rpk")
                        with nc.allow_low_precision(reason="bf16 1/den: ~0.2% softmax scale error, within tolerance"):
                            nc.vector.reciprocal(rpk, dpk)
                        drec = drp.tile([1, QB], BF16, tag="drec")
                        nc.sync.dma_start(
                            out=drec.rearrange("a (p f) -> (a p) f", p=128),
                            in_=rpk,
                        )
                        recb = denp.tile([64, QB], BF16, tag="recb")
                        nc.sync.dma_start(
                            out=recb,
                            in_=bass.AP(
                                tensor=drec.tensor,
                                offset=drec.offset,
                                ap=[[0, 64]] + [list(a) for a in drec.ap[1:]],
                            ),
                        )
                        recbs.append(recb)
                    return recbs

                def emit_outmul(recbs, pvs, fqb, fg, fot2):
                    fqsl = slice(fqb * QB, (fqb + 1) * QB)
                    # pair A: lane-aligned direct write into ot2 rows 0-63
                    nc.vector.tensor_mul(
                        fot2[0:64, fqsl], pvs[0][0:64, :], recbs[0]
                    )
                    # pair B: psum rows 0-63 -> ot2 rows 64-127 via DMA shift
                    otb = otbp.tile([64, QB], BF16, tag="otb")
                    nc.vector.tensor_mul(otb, pvs[1][0:64, :], recbs[1])
                    nc.scalar.dma_start(out=fot2[64:128, fqsl], in_=otb)

                for g in range(2):
                    xt_g = const.tile([128, S], F32, tag="xt", name=f"xt_g{g}_r{rep}")
                    nc.sync.dma_start(
                        out=xt_g.bitcast(F32R),
                        in_=xt_d[g * 128 : (g + 1) * 128, :].bitcast(F32R),
                    )
                    xtb_g = const.tile([128, S], BF16, tag="xtb", name=f"xtb_g{g}_r{rep}")
                    nc.scalar.dma_start(
                        out=xtb_g, in_=xtb_d[g * 128 : (g + 1) * 128, :]
                    )
                    # ---- Q/K projections: row+col tiled (T0 / T10) ----
                    qt = qkp.tile([128, S], BF16, tag="qt", name=f"qt_g{g}_r{rep}")
                    kt = qkp.tile([128, S], BF16, tag="kt", name=f"kt_g{g}_r{rep}")
                    for sp in range(S // 512):
                        ssl = slice(sp * 512, (sp + 1) * 512)
                        for dst, wcol, b_sb in ((qt, 0, bq_sb), (kt, 64, bk_sb)):
                            ps = simp.tile([128, 512], F32, tag="sim")
                            nc.tensor.matmul(
                                ps[0:64, :],
                                wqkb_sb[0:64, wcol : wcol + 64],
                                xtb_g[0:64, ssl],
                                start=True, stop=True,
                                tile_position=(0, 0),
                            )
                            nc.tensor.matmul(
                                ps[64:128, :],
                                wqkb_sb[64:128, wcol : wcol + 64],
                                xtb_g[64:128, ssl],
                                start=True, stop=True,
                                tile_position=(64, 64),
                            )
                            nc.scalar.activation(
                                dst[:, ssl], ps[:, :], Ident, bias=b_sb[0:128, :]
                            )

                    # V tiles ([V | 1] layout); projections interleave into qb 0
                    vts = []
                    for p01 in range(2):
                        vt = vtp.tile(
                            [128, NKC * 65], BF16, tag=f"vt{p01}",
                            name=f"v_g{g}_{p01}_r{rep}",
                        )
                        ones_ap = vt.rearrange("p (c k) -> p c k", k=65)[:, :, 64:65]
                        nc.gpsimd.memset(ones_ap, 1.0)
                        vts.append(vt)

                    ot2 = ot2p.tile([128, S], BF16, tag="ot2", name=f"ot2_g{g}_r{rep}")
                    ot2s.append(ot2)

                    for qb in range(S // QB):
                        qsl = slice(qb * QB, (qb + 1) * QB)
                        pts = [
                            ptp.tile([128, NKC * QB], BF16, tag=f"pt{p}",
                                     name=f"pt{p}_g{g}_q{qb}_r{rep}")
                            for p in range(2)
                        ]
                        if prev is not None:
                            pvs_prev = [
                                accp.tile([65, QB], F32, tag="acc",
                                          name=f"pv{p}_g{g}_q{qb}m1_r{rep}")
                                for p in range(2)
                            ]
                        ctmps = [None, None]
                        vps = None
                        for kc in range(NKC):
                            kc2 = kc // 2
                            j2 = kc % 2
                            plan = PLAN[(qb, kc2 // 2)]
                            # PV for the previous q-block first: its operands are
                            # always ready, so the PE never idles here
                            if prev is not None:
                                for p01 in range(2):
                                    nc.tensor.matmul(
                                        pvs_prev[p01][:, :],
                                        prev["vts"][p01][:, kc * 65 : (kc + 1) * 65],
                                        prev["pts"][p01][:, kc * QB : (kc + 1) * QB],
                                        start=(kc == 0),
                                        stop=(kc == NKC - 1),
                                    )
                            sims = [simp.tile([128, 512], F32, tag="sim", name=f"sim{p}_g{g}_q{qb}_k{kc}_r{rep}") for p in range(2)]
                            for p01 in range(2):
                                rsl = slice(p01 * 64, p01 * 64 + 64)
                                nc.tensor.matmul(
                                    sims[p01][:, :],
                                    kt[rsl, kc * 128 : (kc + 1) * 128],
                                    qt[rsl, qsl],
                                    start=True, stop=True,
                                    tile_position=(p01 * 64, 0),
                                )
                            if qb == 0:
                                # V projection chunk sc=kc (row-tiled pair)
                                if kc % 4 == 0:
                                    vps = [simp.tile([128, 256], F32, tag="sim",
                                                     name=f"vps{p}_g{g}_c{kc // 4}_r{rep}")
                                           for p in range(2)]
                                j = kc % 4
                                for p01 in range(2):
                                    rsl = slice(p01 * 64, p01 * 64 + 64)
                                    nc.tensor.matmul(
                                        vps[p01][:, j * 64 : (j + 1) * 64],
                                        xt_g[rsl, kc * 128 : (kc + 1) * 128].bitcast(F32R),
                                        wv_sb[rsl, :].bitcast(F32R),
                                        start=True, stop=True,
                                        tile_position=(p01 * 64, 0),
                                    )
                                if kc % 4 == 3:
                                    vc4 = kc // 4
                                    for p01 in range(2):
                                        dst = vts[p01].rearrange("p (c k) -> p c k", k=65)[
                                            :, vc4 * 4 : (vc4 + 1) * 4, 0:64
                                        ]
                                        src = vps[p01].rearrange("p (c k) -> p c k", k=64)
                                        if p01 == 0:
                                            nc.scalar.activation(dst, src, Copy)
                                        else:
                                            nc.vector.tensor_copy(dst, src)
                            moff = (kc2 * (S // QB) + qb) * 1024 + j2 * 512
                            span = slice(kc * QB, (kc + 1) * QB)
                            for p01 in range(2):
                                if plan == "C":
                                    if j2 == 0:
                                        ctmps[p01] = ctp.tile(
                                            [128, 1024], BF16, tag=f"ct{p01}",
                                            name=f"ct{p01}_g{g}_q{qb}_k{kc}_r{rep}",
                                        )
                                    nc.scalar.activation(
                                        ctmps[p01][:, j2 * 512 : (j2 + 1) * 512],
                                        sims[p01][:, :], Copy,
                                    )
                                    if j2 == 1:
                                        moff2 = (kc2 * (S // QB) + qb) * 1024
                                        span2k = slice((kc - 1) * QB, (kc + 1) * QB)
                                        nc.gpsimd.tensor_mul(
                                            pts[p01][:, span2k], ctmps[p01],
                                            mask_sb[:, moff2 : moff2 + 1024],
                                        )
                                else:
                                    nc.vector.tensor_mul(
                                        pts[p01][:, span], sims[p01][:, :],
                                        mask_sb[:, moff : moff + 512],
                                    )
                            if kc % 4 == 3:
                                span2 = slice((kc - 3) * QB, (kc + 1) * QB)
                                for p01 in range(2):
                                    seg = pts[p01][:, span2]
                                    if plan == "A":
                                        nc.scalar.activation(seg, seg, Exp)
                                    else:  # B, C -> DVE fast-exp add
                                        nc.vector.tensor_scalar(
                                            seg.bitcast(I16), seg, float(B16), None,
                                            op0=Add,
                                        )
                            if kc == 5 and out_pending is not None:
                                wo_g, wo_qb = out_pending[3], out_pending[2]
                                emit_outmul(*out_pending)
                                out_pending = None
                                if wo_g == 1:
                                    wo_pending = list(range(4 * wo_qb, 4 * wo_qb + 4))
                            if kc in (8, 10, 12, 14) and wo_pending:
                                emit_wo(wo_pending.pop(0))
                        while wo_pending:
                            emit_wo(wo_pending.pop(0))
                        if prev is not None:
                            out_pending = (emit_chain(pvs_prev), pvs_prev,
                                           prev["qb"], prev["g"], prev["ot2"])
                        prev = {"pts": pts, "vts": vts, "ot2": ot2, "qb": qb, "g": g}

                # ---- tail: PV + finish for (g=1, qb=3) ----
                pvs_last = [
                    accp.tile([65, QB], F32, tag="acc", name=f"pv{p}_last_r{rep}")
                    for p in range(2)
                ]
                for c in range(NKC):
                    for p01 in range(2):
                        nc.tensor.matmul(
                            pvs_last[p01][:, :],
                            prev["vts"][p01][:, c * 65 : (c + 1) * 65],
                            prev["pts"][p01][:, c * QB : (c + 1) * QB],
                            start=(c == 0),
                            stop=(c == NKC - 1),
                        )
                if out_pending is not None:
                    emit_outmul(*out_pending)
                    wo_pending = list(range(8, 12))
                ch_last = emit_chain(pvs_last)
                while wo_pending:
                    emit_wo(wo_pending.pop(0))
                emit_outmul(ch_last, pvs_last, 3, 1, ot2s[1])

                # ---- output projection (remaining spans) ----
                for qc in range(12, 16):
                    emit_wo(qc)
    nc.finalize()
    return nc


def _build_runner(repeat=1):
    """Compile once. Returns an object with prep/exec/reduce/run (see use
    in kernel() and test.py)."""
    import jax
    import jax.numpy as jnp
    import numpy as _np
    from jax.experimental.shard_map import shard_map
    from jax.sharding import Mesh, NamedSharding, PartitionSpec

    from concourse import mybir
    from concourse.bass2jax import (
        _bass_exec_p,
        install_neuronx_cc_hook,
        partition_id_tensor,
    )

    nc = _build_nc(repeat=repeat)
    install_neuronx_cc_hook()
    partition_name = nc.partition_id_tensor.name if nc.partition_id_tensor else None

    replicated = {"maskt", "wpack", "wqkb"}

    in_names, out_names, out_avals, out_shapes, out_dtypes = [], [], [], [], []
    for alloc in nc.m.functions[0].allocations:
        if not isinstance(alloc, mybir.MemoryLocationSet):
            continue
        name = alloc.memorylocations[0].name
        if alloc.kind == "ExternalInput":
            if name != partition_name:
                in_names.append(name)
        elif alloc.kind == "ExternalOutput":
            out_names.append(name)
            shape = tuple(alloc.tensor_shape)
            dtype = mybir.dt.np(alloc.dtype)
            out_avals.append(jax.core.ShapedArray(shape, dtype))
            out_shapes.append(shape)
            out_dtypes.append(dtype)

    n_params = len(in_names)
    n_outs = len(out_names)
    all_in_names = list(in_names) + list(out_names)
    if partition_name is not None:
        all_in_names.append(partition_name)
    donate = tuple(range(n_params, n_params + n_outs))

    def _body(*args):
        operands = list(args)
        if partition_name is not None:
            operands.append(partition_id_tensor())
        outs = _bass_exec_p.bind(
            *operands,
            out_avals=tuple(out_avals),
            in_names=tuple(all_in_names),
            out_names=tuple(out_names),
            lowering_input_output_aliases=(),
            sim_require_finite=True,
            sim_require_nnan=True,
            nc=nc,
        )
        return tuple(outs)

    devices = jax.devices()[:N_CORES]
    mesh = Mesh(_np.asarray(devices), ("core",))
    shard0 = NamedSharding(mesh, PartitionSpec("core"))
    srepl = NamedSharding(mesh, PartitionSpec())
    in_specs = tuple(
        PartitionSpec() if name in replicated else PartitionSpec("core")
        for name in in_names
    ) + (PartitionSpec("core"),) * n_outs
    out_specs = (PartitionSpec("core"),) * n_outs

    sharded = jax.jit(
        shard_map(
            _body, mesh=mesh, in_specs=in_specs, out_specs=out_specs,
            check_rep=False,
        ),
        donate_argnums=donate,
        keep_unused=True,
    )

    _zeros = jax.jit(
        lambda: tuple(
            jnp.zeros((N_CORES * s[0], *s[1:]), d)
            for s, d in zip(out_shapes, out_dtypes)
        ),
        out_shardings=(shard0,) * n_outs,
    )

    _reduce = jax.jit(
        lambda p: p.reshape(B, 4, S, E).sum(axis=1).reshape(B * S, E),
        out_shardings=shard0,
    )

    def prep(in_maps):
        args = []
        for name in in_names:
            if name in replicated:
                arr = _np.asarray(in_maps[0][name])
                args.append(jax.device_put(arr, srepl))
            else:
                arr = _np.concatenate(
                    [_np.asarray(m[name]) for m in in_maps], axis=0
                )
                args.append(jax.device_put(arr, shard0))
        return args

    def make_zeros():
        return _zeros()

    def exec_device(args, zeros=None):
        if zeros is None:
            zeros = _zeros()
        outs = sharded(*args, *zeros)
        return jax.block_until_ready(outs[0])

    def exec_async(args, zeros):
        return sharded(*args, *zeros)[0]

    def reduce_device(partials):
        return jax.block_until_ready(_reduce(partials))

    def run(in_maps):
        partials = exec_device(prep(in_maps))
        return _np.asarray(reduce_device(partials))  # (B*S, E)

    class R:
        pass

    r = R()
    r.nc = nc
    r.prep = prep
    r.make_zeros = make_zeros
    r.exec_device = exec_device
    r.exec_async = exec_async
    r.reduce_device = reduce_device
    r.run = run
    return r


def _runtime(repeat=1):
    if repeat not in _RUNTIME:
        _RUNTIME[repeat] = _build_runner(repeat=repeat)
    return _RUNTIME[repeat]


def make_in_maps(x, mask, Wq, bq, Wk, bk, Wv, bv, Wo, bo):
    bf16 = ml_dtypes.bfloat16
    x = np.asarray(x, np.float32)
    m = np.asarray(mask, np.float32).T  # [k, q]
    # device layout: [128, (kc2, qb, j, ql)]; each mul reads one flat
    # [128, 1024] span at moff=(kc2*4+qb)*1024
    maskT = np.ascontiguousarray(
        m.reshape(NKC // 2, 2, 128, S // QB, QB)
        .transpose(2, 0, 3, 1, 4)
        .reshape(128, NKC * S)
    ).astype(np.float32)
    # scale fast-exp regions by A16 (Schraudolph): all plans except 'A'
    mview = maskT.reshape(128, NKC // 2, S // QB, 2 * QB)
    for kc2 in range(NKC // 2):
        for qb in range(S // QB):
            if PLAN[(qb, kc2 // 2)] != "A":
                mview[:, kc2, qb, :] *= np.float32(A16)
    maskT = maskT.astype(bf16)

    wq_s = (np.asarray(Wq, np.float32) * SCALE).astype(np.float32)
    bq_s = (np.asarray(bq, np.float32) * SCALE).astype(np.float32)
    wq2 = np.concatenate([wq_s, wq_s], axis=0)
    wk2 = np.concatenate([np.asarray(Wk, np.float32)] * 2, axis=0)
    wv2 = np.concatenate([np.asarray(Wv, np.float32)] * 2, axis=0)
    bq2 = np.concatenate([bq_s, bq_s])[:, None].astype(np.float32)
    bk2 = np.concatenate([np.asarray(bk, np.float32)] * 2)[:, None].astype(np.float32)
    wpack = np.ascontiguousarray(
        np.concatenate([wq2, wk2, wv2, bq2, bk2], axis=1), np.float32
    )
    wqkb = np.ascontiguousarray(
        np.concatenate([wq2, wk2], axis=1)
    ).astype(bf16)

    in_maps = []
    for c in range(N_CORES):
        b = c // 4
        h0 = (c % 4) * HPC
        r0 = h0 * HD
        xt = np.ascontiguousarray(x[b].T[r0 : r0 + HPC * HD, :])
        xtb = xt.astype(bf16)
        wo = np.ascontiguousarray(np.asarray(Wo, np.float32)[r0 : r0 + HPC * VD, :]).astype(bf16)
        in_maps.append(
            {
                "xt": xt,
                "xtb": xtb,
                "maskt": maskT,
                "wpack": wpack,
                "wqkb": wqkb,
                "wo": wo,
            }
        )
    return in_maps


def kernel(x, mask, Wq, bq, Wk, bk, Wv, bv, Wo, bo):
    r = _runtime()
    in_maps = make_in_maps(x, mask, Wq, bq, Wk, bk, Wv, bv, Wo, bo)
    flat = r.run(in_maps)  # (B*S, E), per-batch partials already summed
    Wo32 = np.asarray(Wo, np.float32)
    crow = np.asarray(bo, np.float32) + np.tile(np.asarray(bv, np.float32), H) @ Wo32
    out = flat.reshape(B, S, E) + crow[None, None, :]
    return out.astype(np.float32)


# revision 17
# speedup vs baseline: 1.1186x; 1.1179x over previous
"""Multi-head attention (B=2, S=2048, E=1024, H=16) on 8 TRN2 NeuronCores.

Sharding: batch x head-group. Core c handles batch c//4 and heads
(c%4)*4 .. +3, as 2 groups x 2 head-pairs. Pair A lives in SBUF/array
partitions 0-63, pair B in 64-127, enabling 2x row/col-tiled matmuls
(64-row PE tiles T0/T8) for the QK product and all projections.

Per (qb, kc4) block of the attention matrix, one of four elementwise
plans computes pt = exp(sim*mask):
  A: DVE mul (psum x mask -> bf16) + ACT true exp (in-place sbuf)
  B: DVE mul (mask pre-scaled by A16) + DVE tensor_scalar add B16 ->
     int16 bits == bf16(exp) (Schraudolph fast-exp; max ~4e-3 final err)
  G: DVE mul + GpSimd tensor_scalar (fast-exp on Q7)
  C: ACT copy psum->sbuf + GpSimd mul + DVE tensor_scalar (fast-exp)
The mix balances DVE/ACT/GpSimd occupancy; the mask tensor is
region-scaled host-side (x A16 for fast-exp regions).

PV keeps the ones-column trick ([V | 1] stationary, 65-col output whose
row 64 is the softmax denominator) and is software-pipelined one q-block
behind QK so the PE never stalls on the elementwise pipeline. The Wo
contraction for q-span qb is emitted inside group 1's attention loop as
soon as both groups' outT spans are ready, hiding the output-projection
tail. Reciprocal via DRAM-bounce repack as before."""
import sys

if "/opt/trn_rl_repo" not in sys.path:
    sys.path.insert(0, "/opt/trn_rl_repo")

from contextlib import ExitStack

import ml_dtypes
import numpy as np

B, S, E = 2, 2048, 1024
H = 16
HD = 64
KD = 64
VD = 64
SCALE = 1.0 / np.float32(np.sqrt(np.float32(KD)))
N_CORES = 8
HPC = H // 4  # heads per core = 4
QB = 512  # q-block width
NKC = S // 128  # 16 k-chunks
A16 = np.float64(128.0 / np.log(2.0))  # Schraudolph bf16 scale
B16 = np.float64(127.0 * 128.0 - 7.4)  # Schraudolph bf16 offset

# plan per (qb, kc4) cell; kc4 = kc2//2 indexes [128, 2048] spans.
# 'A' true exp; 'B' DVE fast-exp; 'G' GpSimd fast-exp; 'C' GpSimd mul.
PLAN = {
    (0, 0): "A", (0, 1): "A", (0, 2): "A", (0, 3): "A",
    (1, 0): "A", (1, 1): "C", (1, 2): "A", (1, 3): "A",
    (2, 0): "A", (2, 1): "A", (2, 2): "A", (2, 3): "A",
    (3, 0): "A", (3, 1): "C", (3, 2): "A", (3, 3): "A",
}

_RUNTIME = {}


def _build_nc(repeat=1):
    import concourse.bass as bass
    import concourse.tile as tile
    from concourse import mybir, bacc

    F32 = mybir.dt.float32
    F32R = mybir.dt.float32r
    BF16 = mybir.dt.bfloat16
    I16 = mybir.dt.int16
    Copy = mybir.ActivationFunctionType.Copy
    Ident = mybir.ActivationFunctionType.Identity
    Exp = mybir.ActivationFunctionType.Exp
    Add = mybir.AluOpType.add

    nc = bacc.Bacc("TRN2")
    mask_d = nc.dram_tensor("maskt", (128, NKC * S), BF16, kind="ExternalInput")
    wp_d = nc.dram_tensor("wpack", (128, 3 * 64 + 2), F32, kind="ExternalInput")
    wqkb_d = nc.dram_tensor("wqkb", (128, 192), BF16, kind="ExternalInput")
    xtb_d = nc.dram_tensor("xtb", (4 * HD, S), BF16, kind="ExternalInput")
    wo_d = nc.dram_tensor("wo", (4 * VD, E), BF16, kind="ExternalInput")
    out_d = nc.dram_tensor("partial", (S, E), BF16, kind="ExternalOutput")

    with tile.TileContext(nc) as tc:
        with ExitStack() as ctx:
            const = ctx.enter_context(tc.tile_pool(name="const", bufs=1))
            qkp = ctx.enter_context(tc.tile_pool(name="qkp", bufs=2))
            vtp = ctx.enter_context(tc.tile_pool(name="vtp", bufs=2))
            ptp = ctx.enter_context(tc.tile_pool(name="ptp", bufs=2))
            ctp = ctx.enter_context(tc.tile_pool(name="ctp", bufs=2))
            otbp = ctx.enter_context(tc.tile_pool(name="otbp", bufs=2))
            ot2p = ctx.enter_context(tc.tile_pool(name="ot2p", bufs=2))
            denp = ctx.enter_context(tc.tile_pool(name="denp", bufs=2))
            wst = ctx.enter_context(tc.tile_pool(name="wst", bufs=2))
            drp = ctx.enter_context(tc.tile_pool(name="drp", bufs=2, space="DRAM"))
            simp = ctx.enter_context(tc.tile_pool(name="simp", bufs=2, space="PSUM"))
            accp = ctx.enter_context(tc.tile_pool(name="accp", bufs=4, space="PSUM"))

            # ---- constant loads ----
            wp_sb = const.tile([128, 3 * 64 + 2], F32, tag="wp")
            nc.sync.dma_start(out=wp_sb.bitcast(F32R), in_=wp_d[:, :].bitcast(F32R))
            wq_sb = wp_sb[:, 0:64]
            wk_sb = wp_sb[:, 64:128]
            wv_sb = wp_sb[:, 128:192]
            bq_sb = wp_sb[:, 192:193]
            bk_sb = wp_sb[:, 193:194]
            mask_sb = const.tile([128, NKC * S], BF16, tag="mask")
            for kc in range(NKC):
                eng = nc.scalar if kc % 2 == 0 else nc.sync
                eng.dma_start(
                    out=mask_sb[:, kc * S : (kc + 1) * S],
                    in_=mask_d[:, kc * S : (kc + 1) * S],
                )
            wqkb_sb = const.tile([128, 192], BF16, tag="wqkb")
            nc.sync.dma_start(out=wqkb_sb, in_=wqkb_d[:, :])
            wo_sb = []
            for g in range(2):
                t = const.tile([128, E], BF16, tag=f"wo{g}")
                nc.scalar.dma_start(out=t, in_=wo_d[g * 128 : (g + 1) * 128, :])
                wo_sb.append(t)

            def emit_wo(qc):
                ost = wst.tile([128, E], BF16, tag="wst", name=f"ost_{qc}")
                for e2 in range(2):
                    wo_ps = simp.tile([128, 512], F32, tag="sim", name=f"wops_{qc}_{e2}")
                    for gi in range(2):
                        nc.tensor.matmul(
                            wo_ps[:, :],
                            ot2s[gi][:, qc * 128 : (qc + 1) * 128],
                            wo_sb[gi][:, e2 * 512 : (e2 + 1) * 512],
                            start=(gi == 0),
                            stop=(gi == 1),
                        )
                    dst = ost[:, e2 * 512 : (e2 + 1) * 512]
                    if (qc + e2) % 2 == 0:
                        nc.scalar.activation(dst, wo_ps[:, :], Copy)
                    else:
                        nc.vector.tensor_copy(dst, wo_ps[:, :])
                eng = nc.sync if qc % 2 == 0 else nc.scalar
                eng.dma_start(out=out_d[qc * 128 : (qc + 1) * 128, :], in_=ost)

            for rep in range(repeat):
                ot2s = []
                # pipeline state carried across group boundaries
                prev = None         # dict(pts, vts, ot2, qb) awaiting PV
                out_pending = None  # (recbs, pvs, qb, g, ot2) awaiting out-mul
                wo_pending = []     # qc indices ready for output projection

                def emit_chain(pvs):
                    """Start denominator -> reciprocal -> broadcast chain."""
                    recbs = []
                    for p01 in range(2):
                        den = denp.tile([1, QB], F32, tag="den")
                        nc.scalar.activation(den, pvs[p01][64:65, :], Copy)
                        dden = drp.tile([1, QB], F32, tag="dden")
                        nc.sync.dma_start(out=dden, in_=den)
                        dpk = denp.tile([128, QB // 128], F32, tag="dpk")
                        nc.sync.dma_start(
                            out=dpk,
                            in_=dden.rearrange("a (p f) -> (a p) f", p=128),
                        )
                        rpk = denp.tile([128, QB // 128], BF16, tag="rpk")
                        with nc.allow_low_precision(reason="bf16 1/den: ~0.2% softmax scale error, within tolerance"):
                            nc.vector.reciprocal(rpk, dpk)
                        drec = drp.tile([1, QB], BF16, tag="drec")
                        nc.sync.dma_start(
                            out=drec.rearrange("a (p f) -> (a p) f", p=128),
                            in_=rpk,
                        )
                        recb = denp.tile([64, QB], BF16, tag="recb")
                        nc.sync.dma_start(
                            out=recb,
                            in_=bass.AP(
                                tensor=drec.tensor,
                                offset=drec.offset,
                                ap=[[0, 64]] + [list(a) for a in drec.ap[1:]],
                            ),
                        )
                        recbs.append(recb)
                    return recbs

                def emit_outmul(recbs, pvs, fqb, fg, fot2):
                    fqsl = slice(fqb * QB, (fqb + 1) * QB)
                    # pair A: lane-aligned direct write into ot2 rows 0-63
                    nc.vector.tensor_mul(
                        fot2[0:64, fqsl], pvs[0][0:64, :], recbs[0]
                    )
                    # pair B: psum rows 0-63 -> ot2 rows 64-127 via DMA shift
                    otb = otbp.tile([64, QB], BF16, tag="otb")
                    nc.vector.tensor_mul(otb, pvs[1][0:64, :], recbs[1])
                    nc.scalar.dma_start(out=fot2[64:128, fqsl], in_=otb)

                for g in range(2):
                    xtb_g = const.tile([128, S], BF16, tag="xtb", name=f"xtb_g{g}_r{rep}")
                    eng = nc.sync if g == 0 else nc.scalar
                    eng.dma_start(
                        out=xtb_g, in_=xtb_d[g * 128 : (g + 1) * 128, :]
                    )
                    # ---- Q/K projections: row+col tiled (T0 / T10) ----
                    qt = qkp.tile([128, S], BF16, tag="qt", name=f"qt_g{g}_r{rep}")
                    kt = qkp.tile([128, S], BF16, tag="kt", name=f"kt_g{g}_r{rep}")
                    for sp in range(S // 512):
                        ssl = slice(sp * 512, (sp + 1) * 512)
                        for dst, wcol, b_sb in ((qt, 0, bq_sb), (kt, 64, bk_sb)):
                            ps = simp.tile([128, 512], F32, tag="sim")
                            nc.tensor.matmul(
                                ps[0:64, :],
                                wqkb_sb[0:64, wcol : wcol + 64],
                                xtb_g[0:64, ssl],
                                start=True, stop=True,
                                tile_position=(0, 0),
                            )
                            nc.tensor.matmul(
                                ps[64:128, :],
                                wqkb_sb[64:128, wcol : wcol + 64],
                                xtb_g[64:128, ssl],
                                start=True, stop=True,
                                tile_position=(64, 64),
                            )
                            nc.scalar.activation(
                                dst[:, ssl], ps[:, :], Ident, bias=b_sb[0:128, :]
                            )

                    # V tiles ([V | 1] layout); projections interleave into qb 0
                    vts = []
                    for p01 in range(2):
                        vt = vtp.tile(
                            [128, NKC * 65], BF16, tag=f"vt{p01}",
                            name=f"v_g{g}_{p01}_r{rep}",
                        )
                        ones_ap = vt.rearrange("p (c k) -> p c k", k=65)[:, :, 64:65]
                        nc.gpsimd.memset(ones_ap, 1.0)
                        vts.append(vt)

                    ot2 = ot2p.tile([128, S], BF16, tag="ot2", name=f"ot2_g{g}_r{rep}")
                    ot2s.append(ot2)

                    for qb in range(S // QB):
                        qsl = slice(qb * QB, (qb + 1) * QB)
                        pts = [
                            ptp.tile([128, NKC * QB], BF16, tag=f"pt{p}",
                                     name=f"pt{p}_g{g}_q{qb}_r{rep}")
                            for p in range(2)
                        ]
                        if prev is not None:
                            pvs_prev = [
                                accp.tile([65, QB], F32, tag="acc",
                                          name=f"pv{p}_g{g}_q{qb}m1_r{rep}")
                                for p in range(2)
                            ]
                        ctmps = [None, None]
                        vps = None
                        for kc in range(NKC):
                            kc2 = kc // 2
                            j2 = kc % 2
                            plan = PLAN[(qb, kc2 // 2)]
                            # PV for the previous q-block first: its operands are
                            # always ready, so the PE never idles here
                            if prev is not None:
                                for p01 in range(2):
                                    nc.tensor.matmul(
                                        pvs_prev[p01][:, :],
                                        prev["vts"][p01][:, kc * 65 : (kc + 1) * 65],
                                        prev["pts"][p01][:, kc * QB : (kc + 1) * QB],
                                        start=(kc == 0),
                                        stop=(kc == NKC - 1),
                                    )
                            sims = [simp.tile([128, 512], F32, tag="sim", name=f"sim{p}_g{g}_q{qb}_k{kc}_r{rep}") for p in range(2)]
                            for p01 in range(2):
                                rsl = slice(p01 * 64, p01 * 64 + 64)
                                nc.tensor.matmul(
                                    sims[p01][:, :],
                                    kt[rsl, kc * 128 : (kc + 1) * 128],
                                    qt[rsl, qsl],
                                    start=True, stop=True,
                                    tile_position=(p01 * 64, 0),
                                )
                            if qb == 0:
                                # V projection chunk sc=kc (row-tiled pair)
                                if kc % 4 == 0:
                                    vps = [simp.tile([128, 256], F32, tag="sim",
                                                     name=f"vps{p}_g{g}_c{kc // 4}_r{rep}")
                                           for p in range(2)]
                                j = kc % 4
                                for p01 in range(2):
                                    rsl = slice(p01 * 64, p01 * 64 + 64)
                                    nc.tensor.matmul(
                                        vps[p01][:, j * 64 : (j + 1) * 64],
                                        xtb_g[rsl, kc * 128 : (kc + 1) * 128],
                                        wqkb_sb[rsl, 128:192],
                                        start=True, stop=True,
                                        tile_position=(p01 * 64, 0),
                                    )
                                if kc % 4 == 3:
                                    vc4 = kc // 4
                                    for p01 in range(2):
                                        dst = vts[p01].rearrange("p (c k) -> p c k", k=65)[
                                            :, vc4 * 4 : (vc4 + 1) * 4, 0:64
                                        ]
                                        src = vps[p01].rearrange("p (c k) -> p c k", k=64)
                                        if p01 == 0:
                                            nc.scalar.activation(dst, src, Copy)
                                        else:
                                            nc.vector.tensor_copy(dst, src)
                            moff = (kc2 * (S // QB) + qb) * 1024 + j2 * 512
                            span = slice(kc * QB, (kc + 1) * QB)
                            for p01 in range(2):
                                if plan == "C":
                                    if j2 == 0:
                                        ctmps[p01] = ctp.tile(
                                            [128, 1024], BF16, tag=f"ct{p01}",
                                            name=f"ct{p01}_g{g}_q{qb}_k{kc}_r{rep}",
                                        )
                                    nc.scalar.activation(
                                        ctmps[p01][:, j2 * 512 : (j2 + 1) * 512],
                                        sims[p01][:, :], Copy,
                                    )
                                    if j2 == 1:
                                        moff2 = (kc2 * (S // QB) + qb) * 1024
                                        span2k = slice((kc - 1) * QB, (kc + 1) * QB)
                                        nc.gpsimd.tensor_mul(
                                            pts[p01][:, span2k], ctmps[p01],
                                            mask_sb[:, moff2 : moff2 + 1024],
                                        )
                                else:
                                    nc.vector.tensor_mul(
                                        pts[p01][:, span], sims[p01][:, :],
                                        mask_sb[:, moff : moff + 512],
                                    )
                            if kc % 4 == 3:
                                span2 = slice((kc - 3) * QB, (kc + 1) * QB)
                                for p01 in range(2):
                                    seg = pts[p01][:, span2]
                                    if plan == "A":
                                        nc.scalar.activation(seg, seg, Exp)
                                    else:  # B, C -> DVE fast-exp add
                                        nc.vector.tensor_scalar(
                                            seg.bitcast(I16), seg, float(B16), None,
                                            op0=Add,
                                        )
                            if kc == 5 and out_pending is not None:
                                wo_g, wo_qb = out_pending[3], out_pending[2]
                                emit_outmul(*out_pending)
                                out_pending = None
                                if wo_g == 1:
                                    wo_pending = list(range(4 * wo_qb, 4 * wo_qb + 4))
                            if kc in (8, 10, 12, 14) and wo_pending:
                                emit_wo(wo_pending.pop(0))
                        while wo_pending:
                            emit_wo(wo_pending.pop(0))
                        if prev is not None:
                            out_pending = (emit_chain(pvs_prev), pvs_prev,
                                           prev["qb"], prev["g"], prev["ot2"])
                        prev = {"pts": pts, "vts": vts, "ot2": ot2, "qb": qb, "g": g}

                # ---- tail: PV + finish for (g=1, qb=3) ----
                pvs_last = [
                    accp.tile([65, QB], F32, tag="acc", name=f"pv{p}_last_r{rep}")
                    for p in range(2)
                ]
                for c in range(NKC):
                    for p01 in range(2):
                        nc.tensor.matmul(
                            pvs_last[p01][:, :],
                            prev["vts"][p01][:, c * 65 : (c + 1) * 65],
                            prev["pts"][p01][:, c * QB : (c + 1) * QB],
                            start=(c == 0),
                            stop=(c == NKC - 1),
                        )
                if out_pending is not None:
                    emit_outmul(*out_pending)
                    wo_pending = list(range(8, 12))
                ch_last = emit_chain(pvs_last)
                while wo_pending:
                    emit_wo(wo_pending.pop(0))
                emit_outmul(ch_last, pvs_last, 3, 1, ot2s[1])

                # ---- output projection (remaining spans) ----
                for qc in range(12, 16):
                    emit_wo(qc)
    nc.finalize()
    return nc


def _build_runner(repeat=1):
    """Compile once. Returns an object with prep/exec/reduce/run (see use
    in kernel() and test.py)."""
    import jax
    import jax.numpy as jnp
    import numpy as _np
    from jax.experimental.shard_map import shard_map
    from jax.sharding import Mesh, NamedSharding, PartitionSpec

    from concourse import mybir
    from concourse.bass2jax import (
        _bass_exec_p,
        install_neuronx_cc_hook,
        partition_id_tensor,
    )

    nc = _build_nc(repeat=repeat)
    install_neuronx_cc_hook()
    partition_name = nc.partition_id_tensor.name if nc.partition_id_tensor else None

    replicated = {"maskt", "wpack", "wqkb"}

    in_names, out_names, out_avals, out_shapes, out_dtypes = [], [], [], [], []
    for alloc in nc.m.functions[0].allocations:
        if not isinstance(alloc, mybir.MemoryLocationSet):
            continue
        name = alloc.memorylocations[0].name
        if alloc.kind == "ExternalInput":
            if name != partition_name:
                in_names.append(name)
        elif alloc.kind == "ExternalOutput":
            out_names.append(name)
            shape = tuple(alloc.tensor_shape)
            dtype = mybir.dt.np(alloc.dtype)
            out_avals.append(jax.core.ShapedArray(shape, dtype))
            out_shapes.append(shape)
            out_dtypes.append(dtype)

    n_params = len(in_names)
    n_outs = len(out_names)
    all_in_names = list(in_names) + list(out_names)
    if partition_name is not None:
        all_in_names.append(partition_name)
    donate = tuple(range(n_params, n_params + n_outs))

    def _body(*args):
        operands = list(args)
        if partition_name is not None:
            operands.append(partition_id_tensor())
        outs = _bass_exec_p.bind(
            *operands,
            out_avals=tuple(out_avals),
            in_names=tuple(all_in_names),
            out_names=tuple(out_names),
            lowering_input_output_aliases=(),
            sim_require_finite=True,
            sim_require_nnan=True,
            nc=nc,
        )
        return tuple(outs)

    devices = jax.devices()[:N_CORES]
    mesh = Mesh(_np.asarray(devices), ("core",))
    shard0 = NamedSharding(mesh, PartitionSpec("core"))
    srepl = NamedSharding(mesh, PartitionSpec())
    in_specs = tuple(
        PartitionSpec() if name in replicated else PartitionSpec("core")
        for name in in_names
    ) + (PartitionSpec("core"),) * n_outs
    out_specs = (PartitionSpec("core"),) * n_outs

    sharded = jax.jit(
        shard_map(
            _body, mesh=mesh, in_specs=in_specs, out_specs=out_specs,
            check_rep=False,
        ),
        donate_argnums=donate,
        keep_unused=True,
    )

    _zeros = jax.jit(
        lambda: tuple(
            jnp.zeros((N_CORES * s[0], *s[1:]), d)
            for s, d in zip(out_shapes, out_dtypes)
        ),
        out_shardings=(shard0,) * n_outs,
    )

    _reduce = jax.jit(
        lambda p: p.reshape(B, 4, S, E).sum(axis=1).reshape(B * S, E),
        out_shardings=shard0,
    )

    def prep(in_maps):
        args = []
        for name in in_names:
            if name in replicated:
                arr = _np.asarray(in_maps[0][name])
                args.append(jax.device_put(arr, srepl))
            else:
                arr = _np.concatenate(
                    [_np.asarray(m[name]) for m in in_maps], axis=0
                )
                args.append(jax.device_put(arr, shard0))
        return args

    def make_zeros():
        return _zeros()

    def exec_device(args, zeros=None):
        if zeros is None:
            zeros = _zeros()
        outs = sharded(*args, *zeros)
        return jax.block_until_ready(outs[0])

    def exec_async(args, zeros):
        return sharded(*args, *zeros)[0]

    def reduce_device(partials):
        return jax.block_until_ready(_reduce(partials))

    def run(in_maps):
        partials = exec_device(prep(in_maps))
        return _np.asarray(reduce_device(partials))  # (B*S, E)

    class R:
        pass

    r = R()
    r.nc = nc
    r.prep = prep
    r.make_zeros = make_zeros
    r.exec_device = exec_device
    r.exec_async = exec_async
    r.reduce_device = reduce_device
    r.run = run
    return r


def _runtime(repeat=1):
    if repeat not in _RUNTIME:
        _RUNTIME[repeat] = _build_runner(repeat=repeat)
    return _RUNTIME[repeat]


def make_in_maps(x, mask, Wq, bq, Wk, bk, Wv, bv, Wo, bo):
    bf16 = ml_dtypes.bfloat16
    x = np.asarray(x, np.float32)
    m = np.asarray(mask, np.float32).T  # [k, q]
    # device layout: [128, (kc2, qb, j, ql)]; each mul reads one flat
    # [128, 1024] span at moff=(kc2*4+qb)*1024
    maskT = np.ascontiguousarray(
        m.reshape(NKC // 2, 2, 128, S // QB, QB)
        .transpose(2, 0, 3, 1, 4)
        .reshape(128, NKC * S)
    ).astype(np.float32)
    # scale fast-exp regions by A16 (Schraudolph): all plans except 'A'
    mview = maskT.reshape(128, NKC // 2, S // QB, 2 * QB)
    for kc2 in range(NKC // 2):
        for qb in range(S // QB):
            if PLAN[(qb, kc2 // 2)] != "A":
                mview[:, kc2, qb, :] *= np.float32(A16)
    maskT = maskT.astype(bf16)

    wq_s = (np.asarray(Wq, np.float32) * SCALE).astype(np.float32)
    bq_s = (np.asarray(bq, np.float32) * SCALE).astype(np.float32)
    wq2 = np.concatenate([wq_s, wq_s], axis=0)
    wk2 = np.concatenate([np.asarray(Wk, np.float32)] * 2, axis=0)
    wv2 = np.concatenate([np.asarray(Wv, np.float32)] * 2, axis=0)
    bq2 = np.concatenate([bq_s, bq_s])[:, None].astype(np.float32)
    bk2 = np.concatenate([np.asarray(bk, np.float32)] * 2)[:, None].astype(np.float32)
    wpack = np.ascontiguousarray(
        np.concatenate([wq2, wk2, wv2, bq2, bk2], axis=1), np.float32
    )
    wqkb = np.ascontiguousarray(
        np.concatenate([wq2, wk2, wv2], axis=1)
    ).astype(bf16)

    in_maps = []
    for c in range(N_CORES):
        b = c // 4
        h0 = (c % 4) * HPC
        r0 = h0 * HD
        xtb = np.ascontiguousarray(x[b].T[r0 : r0 + HPC * HD, :]).astype(bf16)
        wo = np.ascontiguousarray(np.asarray(Wo, np.float32)[r0 : r0 + HPC * VD, :]).astype(bf16)
        in_maps.append(
            {
                "xtb": xtb,
                "maskt": maskT,
                "wpack": wpack,
                "wqkb": wqkb,
                "wo": wo,
            }
        )
    return in_maps


def kernel(x, mask, Wq, bq, Wk, bk, Wv, bv, Wo, bo):
    r = _runtime()
    in_maps = make_in_maps(x, mask, Wq, bq, Wk, bk, Wv, bv, Wo, bo)
    flat = r.run(in_maps)  # (B*S, E), per-batch partials already summed
    Wo32 = np.asarray(Wo, np.float32)
    crow = np.asarray(bo, np.float32) + np.tile(np.asarray(bv, np.float32), H) @ Wo32
    out = flat.reshape(B, S, E) + crow[None, None, :]
    return out.astype(np.float32)


# revision 18
# speedup vs baseline: 1.1247x; 1.0055x over previous
"""Multi-head attention (B=2, S=2048, E=1024, H=16) on 8 TRN2 NeuronCores.

Sharding: batch x head-group. Core c handles batch c//4 and heads
(c%4)*4 .. +3, as 2 groups x 2 head-pairs. Pair A lives in SBUF/array
partitions 0-63, pair B in 64-127, enabling 2x row/col-tiled matmuls
(64-row PE tiles T0/T8) for the QK product and all projections.

Per (qb, kc4) block of the attention matrix, one of four elementwise
plans computes pt = exp(sim*mask):
  A: DVE mul (psum x mask -> bf16) + ACT true exp (in-place sbuf)
  B: DVE mul (mask pre-scaled by A16) + DVE tensor_scalar add B16 ->
     int16 bits == bf16(exp) (Schraudolph fast-exp; max ~4e-3 final err)
  G: DVE mul + GpSimd tensor_scalar (fast-exp on Q7)
  C: ACT copy psum->sbuf + GpSimd mul + DVE tensor_scalar (fast-exp)
The mix balances DVE/ACT/GpSimd occupancy; the mask tensor is
region-scaled host-side (x A16 for fast-exp regions).

PV keeps the ones-column trick ([V | 1] stationary, 65-col output whose
row 64 is the softmax denominator) and is software-pipelined one q-block
behind QK so the PE never stalls on the elementwise pipeline. The Wo
contraction for q-span qb is emitted inside group 1's attention loop as
soon as both groups' outT spans are ready, hiding the output-projection
tail. Reciprocal via DRAM-bounce repack as before."""
import sys

if "/opt/trn_rl_repo" not in sys.path:
    sys.path.insert(0, "/opt/trn_rl_repo")

from contextlib import ExitStack

import ml_dtypes
import numpy as np

B, S, E = 2, 2048, 1024
H = 16
HD = 64
KD = 64
VD = 64
SCALE = 1.0 / np.float32(np.sqrt(np.float32(KD)))
N_CORES = 8
HPC = H // 4  # heads per core = 4
QB = 512  # q-block width
NKC = S // 128  # 16 k-chunks
A16 = np.float64(128.0 / np.log(2.0))  # Schraudolph bf16 scale
B16 = np.float64(127.0 * 128.0 - 7.4)  # Schraudolph bf16 offset

# plan per (qb, kc4) cell; kc4 = kc2//2 indexes [128, 2048] spans.
# 'A' true exp; 'B' DVE fast-exp; 'G' GpSimd fast-exp; 'C' GpSimd mul.
PLAN = {
    (0, 0): "A", (0, 1): "A", (0, 2): "A", (0, 3): "A",
    (1, 0): "A", (1, 1): "C", (1, 2): "A", (1, 3): "A",
    (2, 0): "A", (2, 1): "A", (2, 2): "A", (2, 3): "A",
    (3, 0): "A", (3, 1): "C", (3, 2): "A", (3, 3): "A",
}

_RUNTIME = {}


def _build_nc(repeat=1):
    import concourse.bass as bass
    import concourse.tile as tile
    from concourse import mybir, bacc

    F32 = mybir.dt.float32
    F32R = mybir.dt.float32r
    BF16 = mybir.dt.bfloat16
    I16 = mybir.dt.int16
    Copy = mybir.ActivationFunctionType.Copy
    Ident = mybir.ActivationFunctionType.Identity
    Exp = mybir.ActivationFunctionType.Exp
    Add = mybir.AluOpType.add

    nc = bacc.Bacc("TRN2")
    mask_d = nc.dram_tensor("maskt", (128, NKC * S), BF16, kind="ExternalInput")
    wp_d = nc.dram_tensor("wpack", (128, 3 * 64 + 2), F32, kind="ExternalInput")
    wqkb_d = nc.dram_tensor("wqkb", (128, 192), BF16, kind="ExternalInput")
    xtb_d = nc.dram_tensor("xtb", (4 * HD, S), BF16, kind="ExternalInput")
    wo_d = nc.dram_tensor("wo", (4 * VD, E), BF16, kind="ExternalInput")
    out_d = nc.dram_tensor("partial", (S, E), BF16, kind="ExternalOutput")

    with tile.TileContext(nc) as tc:
        with ExitStack() as ctx:
            const = ctx.enter_context(tc.tile_pool(name="const", bufs=1))
            qkp = ctx.enter_context(tc.tile_pool(name="qkp", bufs=2))
            vtp = ctx.enter_context(tc.tile_pool(name="vtp", bufs=2))
            ptp = ctx.enter_context(tc.tile_pool(name="ptp", bufs=2))
            ctp = ctx.enter_context(tc.tile_pool(name="ctp", bufs=2))
            otbp = ctx.enter_context(tc.tile_pool(name="otbp", bufs=2))
            ot2p = ctx.enter_context(tc.tile_pool(name="ot2p", bufs=2))
            denp = ctx.enter_context(tc.tile_pool(name="denp", bufs=2))
            wst = ctx.enter_context(tc.tile_pool(name="wst", bufs=2))
            drp = ctx.enter_context(tc.tile_pool(name="drp", bufs=2, space="DRAM"))
            simp = ctx.enter_context(tc.tile_pool(name="simp", bufs=2, space="PSUM"))
            accp = ctx.enter_context(tc.tile_pool(name="accp", bufs=4, space="PSUM"))

            # ---- constant loads ----
            wp_sb = const.tile([128, 3 * 64 + 2], F32, tag="wp")
            nc.sync.dma_start(out=wp_sb.bitcast(F32R), in_=wp_d[:, :].bitcast(F32R))
            wq_sb = wp_sb[:, 0:64]
            wk_sb = wp_sb[:, 64:128]
            wv_sb = wp_sb[:, 128:192]
            bq_sb = wp_sb[:, 192:193]
            bk_sb = wp_sb[:, 193:194]
            mask_sb = const.tile([128, NKC * S], BF16, tag="mask")
            for kc in range(NKC):
                eng = nc.scalar if kc % 2 == 0 else nc.sync
                eng.dma_start(
                    out=mask_sb[:, kc * S : (kc + 1) * S],
                    in_=mask_d[:, kc * S : (kc + 1) * S],
                )
            wqkb_sb = const.tile([128, 192], BF16, tag="wqkb")
            nc.sync.dma_start(out=wqkb_sb, in_=wqkb_d[:, :])
            wo_sb = []
            for g in range(2):
                t = const.tile([128, E], BF16, tag=f"wo{g}")
                nc.scalar.dma_start(out=t, in_=wo_d[g * 128 : (g + 1) * 128, :])
                wo_sb.append(t)

            def emit_wo(qc):
                ost = wst.tile([128, E], BF16, tag="wst", name=f"ost_{qc}")
                for e2 in range(2):
                    wo_ps = simp.tile([128, 512], F32, tag="sim", name=f"wops_{qc}_{e2}")
                    for gi in range(2):
                        nc.tensor.matmul(
                            wo_ps[:, :],
                            ot2s[gi][:, qc * 128 : (qc + 1) * 128],
                            wo_sb[gi][:, e2 * 512 : (e2 + 1) * 512],
                            start=(gi == 0),
                            stop=(gi == 1),
                        )
                    dst = ost[:, e2 * 512 : (e2 + 1) * 512]
                    if (qc + e2) % 2 == 0:
                        nc.scalar.activation(dst, wo_ps[:, :], Copy)
                    else:
                        nc.vector.tensor_copy(dst, wo_ps[:, :])
                eng = nc.sync if qc % 2 == 0 else nc.scalar
                eng.dma_start(out=out_d[qc * 128 : (qc + 1) * 128, :], in_=ost)

            for rep in range(repeat):
                ot2s = []
                # pipeline state carried across group boundaries
                prev = None         # dict(pts, vts, ot2, qb) awaiting PV
                out_pending = None  # (recbs, pvs, qb, g, ot2) awaiting out-mul
                wo_pending = []     # qc indices ready for output projection

                def emit_chain(pvs):
                    """Start denominator -> reciprocal -> broadcast chain."""
                    recbs = []
                    for p01 in range(2):
                        den = denp.tile([1, QB], F32, tag="den")
                        nc.scalar.activation(den, pvs[p01][64:65, :], Copy)
                        dden = drp.tile([1, QB], F32, tag="dden")
                        nc.sync.dma_start(out=dden, in_=den)
                        dpk = denp.tile([128, QB // 128], F32, tag="dpk")
                        nc.sync.dma_start(
                            out=dpk,
                            in_=dden.rearrange("a (p f) -> (a p) f", p=128),
                        )
                        rpk = denp.tile([128, QB // 128], BF16, tag="rpk")
                        with nc.allow_low_precision(reason="bf16 1/den: ~0.2% softmax scale error, within tolerance"):
                            nc.vector.reciprocal(rpk, dpk)
                        drec = drp.tile([1, QB], BF16, tag="drec")
                        nc.sync.dma_start(
                            out=drec.rearrange("a (p f) -> (a p) f", p=128),
                            in_=rpk,
                        )
                        recb = denp.tile([64, QB], BF16, tag="recb")
                        nc.sync.dma_start(
                            out=recb,
                            in_=bass.AP(
                                tensor=drec.tensor,
                                offset=drec.offset,
                                ap=[[0, 64]] + [list(a) for a in drec.ap[1:]],
                            ),
                        )
                        recbs.append(recb)
                    return recbs

                def emit_outmul(recbs, pvs, fqb, fg, fot2):
                    fqsl = slice(fqb * QB, (fqb + 1) * QB)
                    # pair A: lane-aligned direct write into ot2 rows 0-63
                    nc.vector.tensor_mul(
                        fot2[0:64, fqsl], pvs[0][0:64, :], recbs[0]
                    )
                    # pair B: psum rows 0-63 -> ot2 rows 64-127 via DMA shift
                    otb = otbp.tile([64, QB], BF16, tag="otb")
                    nc.vector.tensor_mul(otb, pvs[1][0:64, :], recbs[1])
                    nc.scalar.dma_start(out=fot2[64:128, fqsl], in_=otb)

                for g in range(2):
                    xtb_g = const.tile([128, S], BF16, tag="xtb", name=f"xtb_g{g}_r{rep}")
                    eng = nc.sync if g == 0 else nc.scalar
                    eng.dma_start(
                        out=xtb_g, in_=xtb_d[g * 128 : (g + 1) * 128, :]
                    )
                    # ---- Q/K projections: row+col tiled (T0 / T10) ----
                    qt = qkp.tile([128, S], BF16, tag="qt", name=f"qt_g{g}_r{rep}")
                    kt = qkp.tile([128, S], BF16, tag="kt", name=f"kt_g{g}_r{rep}")
                    for sp in range(S // 512):
                        ssl = slice(sp * 512, (sp + 1) * 512)
                        for dst, wcol, b_sb in ((qt, 0, bq_sb), (kt, 64, bk_sb)):
                            ps = simp.tile([128, 512], F32, tag="sim")
                            nc.tensor.matmul(
                                ps[0:64, :],
                                wqkb_sb[0:64, wcol : wcol + 64],
                                xtb_g[0:64, ssl],
                                start=True, stop=True,
                                tile_position=(0, 0),
                            )
                            nc.tensor.matmul(
                                ps[64:128, :],
                                wqkb_sb[64:128, wcol : wcol + 64],
                                xtb_g[64:128, ssl],
                                start=True, stop=True,
                                tile_position=(64, 64),
                            )
                            nc.scalar.activation(
                                dst[:, ssl], ps[:, :], Ident, bias=b_sb[0:128, :]
                            )

                    # V tiles ([V | 1] layout); projections interleave into qb 0
                    vts = []
                    for p01 in range(2):
                        vt = vtp.tile(
                            [128, NKC * 65], BF16, tag=f"vt{p01}",
                            name=f"v_g{g}_{p01}_r{rep}",
                        )
                        ones_ap = vt.rearrange("p (c k) -> p c k", k=65)[:, :, 64:65]
                        nc.gpsimd.memset(ones_ap, 1.0)
                        vts.append(vt)

                    ot2 = ot2p.tile([128, S], BF16, tag="ot2", name=f"ot2_g{g}_r{rep}")
                    ot2s.append(ot2)

                    for qb in range(S // QB):
                        qsl = slice(qb * QB, (qb + 1) * QB)
                        pts = [
                            ptp.tile([128, NKC * QB], BF16, tag=f"pt{p}",
                                     name=f"pt{p}_g{g}_q{qb}_r{rep}")
                            for p in range(2)
                        ]
                        if prev is not None:
                            pvs_prev = [
                                accp.tile([65, QB], F32, tag="acc",
                                          name=f"pv{p}_g{g}_q{qb}m1_r{rep}")
                                for p in range(2)
                            ]
                        ctmps = [None, None]
                        vps = None
                        for kc in range(NKC):
                            kc2 = kc // 2
                            j2 = kc % 2
                            plan = PLAN[(qb, kc2 // 2)]
                            # PV for the previous q-block in 8-matmul bursts:
                            # operands are always ready and the dense burst
                            # keeps the PE activity monitor at full clock
                            if prev is not None and kc % 4 == 0:
                                for c in range(kc, kc + 4):
                                    for p01 in range(2):
                                        nc.tensor.matmul(
                                            pvs_prev[p01][:, :],
                                            prev["vts"][p01][:, c * 65 : (c + 1) * 65],
                                            prev["pts"][p01][:, c * QB : (c + 1) * QB],
                                            start=(c == 0),
                                            stop=(c == NKC - 1),
                                        )
                            sims = [simp.tile([128, 512], F32, tag="sim", name=f"sim{p}_g{g}_q{qb}_k{kc}_r{rep}") for p in range(2)]
                            for p01 in range(2):
                                rsl = slice(p01 * 64, p01 * 64 + 64)
                                nc.tensor.matmul(
                                    sims[p01][:, :],
                                    kt[rsl, kc * 128 : (kc + 1) * 128],
                                    qt[rsl, qsl],
                                    start=True, stop=True,
                                    tile_position=(p01 * 64, 0),
                                )
                            if qb == 0 and kc % 4 == 0:
                                # V projection burst: chunks kc..kc+3, row-tiled
                                vc4 = kc // 4
                                vps = [simp.tile([128, 256], F32, tag="sim",
                                                 name=f"vps{p}_g{g}_c{vc4}_r{rep}")
                                       for p in range(2)]
                                for j in range(4):
                                    sc = kc + j
                                    for p01 in range(2):
                                        rsl = slice(p01 * 64, p01 * 64 + 64)
                                        nc.tensor.matmul(
                                            vps[p01][:, j * 64 : (j + 1) * 64],
                                            xtb_g[rsl, sc * 128 : (sc + 1) * 128],
                                            wqkb_sb[rsl, 128:192],
                                            start=True, stop=True,
                                            tile_position=(p01 * 64, 0),
                                        )
                                for p01 in range(2):
                                    dst = vts[p01].rearrange("p (c k) -> p c k", k=65)[
                                        :, vc4 * 4 : (vc4 + 1) * 4, 0:64
                                    ]
                                    vsrc = vps[p01].rearrange("p (c k) -> p c k", k=64)
                                    if p01 == 0:
                                        nc.scalar.activation(dst, vsrc, Copy)
                                    else:
                                        nc.vector.tensor_copy(dst, vsrc)
                            moff = (kc2 * (S // QB) + qb) * 1024 + j2 * 512
                            span = slice(kc * QB, (kc + 1) * QB)
                            for p01 in range(2):
                                if plan == "C":
                                    if j2 == 0:
                                        ctmps[p01] = ctp.tile(
                                            [128, 1024], BF16, tag=f"ct{p01}",
                                            name=f"ct{p01}_g{g}_q{qb}_k{kc}_r{rep}",
                                        )
                                    nc.scalar.activation(
                                        ctmps[p01][:, j2 * 512 : (j2 + 1) * 512],
                                        sims[p01][:, :], Copy,
                                    )
                                    if j2 == 1:
                                        moff2 = (kc2 * (S // QB) + qb) * 1024
                                        span2k = slice((kc - 1) * QB, (kc + 1) * QB)
                                        nc.gpsimd.tensor_mul(
                                            pts[p01][:, span2k], ctmps[p01],
                                            mask_sb[:, moff2 : moff2 + 1024],
                                        )
                                else:
                                    nc.vector.tensor_mul(
                                        pts[p01][:, span], sims[p01][:, :],
                                        mask_sb[:, moff : moff + 512],
                                    )
                            if kc % 4 == 3:
                                span2 = slice((kc - 3) * QB, (kc + 1) * QB)
                                for p01 in range(2):
                                    seg = pts[p01][:, span2]
                                    if plan == "A":
                                        nc.scalar.activation(seg, seg, Exp)
                                    else:  # B, C -> DVE fast-exp add
                                        nc.vector.tensor_scalar(
                                            seg.bitcast(I16), seg, float(B16), None,
                                            op0=Add,
                                        )
                            if kc == 5 and out_pending is not None:
                                wo_g, wo_qb = out_pending[3], out_pending[2]
                                emit_outmul(*out_pending)
                                out_pending = None
                                if wo_g == 1:
                                    wo_pending = list(range(4 * wo_qb, 4 * wo_qb + 4))
                            if kc in (8, 10, 12, 14) and wo_pending:
                                emit_wo(wo_pending.pop(0))
                        while wo_pending:
                            emit_wo(wo_pending.pop(0))
                        if prev is not None:
                            out_pending = (emit_chain(pvs_prev), pvs_prev,
                                           prev["qb"], prev["g"], prev["ot2"])
                        prev = {"pts": pts, "vts": vts, "ot2": ot2, "qb": qb, "g": g}

                # ---- tail: PV + finish for (g=1, qb=3) ----
                pvs_last = [
                    accp.tile([65, QB], F32, tag="acc", name=f"pv{p}_last_r{rep}")
                    for p in range(2)
                ]
                for c in range(NKC):
                    for p01 in range(2):
                        nc.tensor.matmul(
                            pvs_last[p01][:, :],
                            prev["vts"][p01][:, c * 65 : (c + 1) * 65],
                            prev["pts"][p01][:, c * QB : (c + 1) * QB],
                            start=(c == 0),
                            stop=(c == NKC - 1),
                        )
                if out_pending is not None:
                    emit_outmul(*out_pending)
                    wo_pending = list(range(8, 12))
                ch_last = emit_chain(pvs_last)
                while wo_pending:
                    emit_wo(wo_pending.pop(0))
                emit_outmul(ch_last, pvs_last, 3, 1, ot2s[1])

                # ---- output projection (remaining spans) ----
                for qc in range(12, 16):
                    emit_wo(qc)
    nc.finalize()
    return nc


def _build_runner(repeat=1):
    """Compile once. Returns an object with prep/exec/reduce/run (see use
    in kernel() and test.py)."""
    import jax
    import jax.numpy as jnp
    import numpy as _np
    from jax.experimental.shard_map import shard_map
    from jax.sharding import Mesh, NamedSharding, PartitionSpec

    from concourse import mybir
    from concourse.bass2jax import (
        _bass_exec_p,
        install_neuronx_cc_hook,
        partition_id_tensor,
    )

    nc = _build_nc(repeat=repeat)
    install_neuronx_cc_hook()
    partition_name = nc.partition_id_tensor.name if nc.partition_id_tensor else None

    replicated = {"maskt", "wpack", "wqkb"}

    in_names, out_names, out_avals, out_shapes, out_dtypes = [], [], [], [], []
    for alloc in nc.m.functions[0].allocations:
        if not isinstance(alloc, mybir.MemoryLocationSet):
            continue
        name = alloc.memorylocations[0].name
        if alloc.kind == "ExternalInput":
            if name != partition_name:
                in_names.append(name)
        elif alloc.kind == "ExternalOutput":
            out_names.append(name)
            shape = tuple(alloc.tensor_shape)
            dtype = mybir.dt.np(alloc.dtype)
            out_avals.append(jax.core.ShapedArray(shape, dtype))
            out_shapes.append(shape)
            out_dtypes.append(dtype)

    n_params = len(in_names)
    n_outs = len(out_names)
    all_in_names = list(in_names) + list(out_names)
    if partition_name is not None:
        all_in_names.append(partition_name)
    donate = tuple(range(n_params, n_params + n_outs))

    def _body(*args):
        operands = list(args)
        if partition_name is not None:
            operands.append(partition_id_tensor())
        outs = _bass_exec_p.bind(
            *operands,
            out_avals=tuple(out_avals),
            in_names=tuple(all_in_names),
            out_names=tuple(out_names),
            lowering_input_output_aliases=(),
            sim_require_finite=True,
            sim_require_nnan=True,
            nc=nc,
        )
        return tuple(outs)

    devices = jax.devices()[:N_CORES]
    mesh = Mesh(_np.asarray(devices), ("core",))
    shard0 = NamedSharding(mesh, PartitionSpec("core"))
    srepl = NamedSharding(mesh, PartitionSpec())
    in_specs = tuple(
        PartitionSpec() if name in replicated else PartitionSpec("core")
        for name in in_names
    ) + (PartitionSpec("core"),) * n_outs
    out_specs = (PartitionSpec("core"),) * n_outs

    sharded = jax.jit(
        shard_map(
            _body, mesh=mesh, in_specs=in_specs, out_specs=out_specs,
            check_rep=False,
        ),
        donate_argnums=donate,
        keep_unused=True,
    )

    _zeros = jax.jit(
        lambda: tuple(
            jnp.zeros((N_CORES * s[0], *s[1:]), d)
            for s, d in zip(out_shapes, out_dtypes)
        ),
        out_shardings=(shard0,) * n_outs,
    )

    _reduce = jax.jit(
        lambda p: p.reshape(B, 4, S, E).sum(axis=1).reshape(B * S, E),
        out_shardings=shard0,
    )

    def prep(in_maps):
        args = []
        for name in in_names:
            if name in replicated:
                arr = _np.asarray(in_maps[0][name])
                args.append(jax.device_put(arr, srepl))
            else:
                arr = _np.concatenate(
                    [_np.asarray(m[name]) for m in in_maps], axis=0
                )
                args.append(jax.device_put(arr, shard0))
        return args

    def make_zeros():
        return _zeros()

    def exec_device(args, zeros=None):
        if zeros is None:
            zeros = _zeros()
        outs = sharded(*args, *zeros)
        return jax.block_until_ready(outs[0])

    def exec_async(args, zeros):
        return sharded(*args, *zeros)[0]

    def reduce_device(partials):
        return jax.block_until_ready(_reduce(partials))

    def run(in_maps):
        partials = exec_device(prep(in_maps))
        return _np.asarray(reduce_device(partials))  # (B*S, E)

    class R:
        pass

    r = R()
    r.nc = nc
    r.prep = prep
    r.make_zeros = make_zeros
    r.exec_device = exec_device
    r.exec_async = exec_async
    r.reduce_device = reduce_device
    r.run = run
    return r


def _runtime(repeat=1):
    if repeat not in _RUNTIME:
        _RUNTIME[repeat] = _build_runner(repeat=repeat)
    return _RUNTIME[repeat]


def make_in_maps(x, mask, Wq, bq, Wk, bk, Wv, bv, Wo, bo):
    bf16 = ml_dtypes.bfloat16
    x = np.asarray(x, np.float32)
    m = np.asarray(mask, np.float32).T  # [k, q]
    # device layout: [128, (kc2, qb, j, ql)]; each mul reads one flat
    # [128, 1024] span at moff=(kc2*4+qb)*1024
    maskT = np.ascontiguousarray(
        m.reshape(NKC // 2, 2, 128, S // QB, QB)
        .transpose(2, 0, 3, 1, 4)
        .reshape(128, NKC * S)
    ).astype(np.float32)
    # scale fast-exp regions by A16 (Schraudolph): all plans except 'A'
    mview = maskT.reshape(128, NKC // 2, S // QB, 2 * QB)
    for kc2 in range(NKC // 2):
        for qb in range(S // QB):
            if PLAN[(qb, kc2 // 2)] != "A":
                mview[:, kc2, qb, :] *= np.float32(A16)
    maskT = maskT.astype(bf16)

    wq_s = (np.asarray(Wq, np.float32) * SCALE).astype(np.float32)
    bq_s = (np.asarray(bq, np.float32) * SCALE).astype(np.float32)
    wq2 = np.concatenate([wq_s, wq_s], axis=0)
    wk2 = np.concatenate([np.asarray(Wk, np.float32)] * 2, axis=0)
    wv2 = np.concatenate([np.asarray(Wv, np.float32)] * 2, axis=0)
    bq2 = np.concatenate([bq_s, bq_s])[:, None].astype(np.float32)
    bk2 = np.concatenate([np.asarray(bk, np.float32)] * 2)[:, None].astype(np.float32)
    wpack = np.ascontiguousarray(
        np.concatenate([wq2, wk2, wv2, bq2, bk2], axis=1), np.float32
    )
    wqkb = np.ascontiguousarray(
        np.concatenate([wq2, wk2, wv2], axis=1)
    ).astype(bf16)

    in_maps = []
    for c in range(N_CORES):
        b = c // 4
        h0 = (c % 4) * HPC
        r0 = h0 * HD
        xtb = np.ascontiguousarray(x[b].T[r0 : r0 + HPC * HD, :]).astype(bf16)
        wo = np.ascontiguousarray(np.asarray(Wo, np.float32)[r0 : r0 + HPC * VD, :]).astype(bf16)
        in_maps.append(
            {
                "xtb": xtb,
                "maskt": maskT,
                "wpack": wpack,
                "wqkb": wqkb,
                "wo": wo,
            }
        )
    return in_maps


def kernel(x, mask, Wq, bq, Wk, bk, Wv, bv, Wo, bo):
    r = _runtime()
    in_maps = make_in_maps(x, mask, Wq, bq, Wk, bk, Wv, bv, Wo, bo)
    flat = r.run(in_maps)  # (B*S, E), per-batch partials already summed
    Wo32 = np.asarray(Wo, np.float32)
    crow = np.asarray(bo, np.float32) + np.tile(np.asarray(bv, np.float32), H) @ Wo32
    out = flat.reshape(B, S, E) + crow[None, None, :]
    return out.astype(np.float32)


# revision 20
# speedup vs baseline: 1.4434x; 1.2833x over previous
"""Multi-head attention (B=2, S=2048, E=1024, H=16) on 8 TRN2 NeuronCores.

Sharding: batch x head-group. Core c handles batch c//4 and heads
(c%4)*4 .. +3, as 2 groups x 2 head-pairs. Pair A lives in SBUF/array
partitions 0-63, pair B in 64-127, enabling 2x row/col-tiled matmuls
(64-row PE tiles T0/T8) for the QK product and all projections.

Per (qb, kc4) block of the attention matrix, one of four elementwise
plans computes pt = exp(sim*mask):
  A: DVE mul (psum x mask -> bf16) + ACT true exp (in-place sbuf)
  B: DVE mul (mask pre-scaled by A16) + DVE tensor_scalar add B16 ->
     int16 bits == bf16(exp) (Schraudolph fast-exp; max ~4e-3 final err)
  G: DVE mul + GpSimd tensor_scalar (fast-exp on Q7)
  C: ACT copy psum->sbuf + GpSimd mul + DVE tensor_scalar (fast-exp)
The mix balances DVE/ACT/GpSimd occupancy; the mask tensor is
region-scaled host-side (x A16 for fast-exp regions).

PV keeps the ones-column trick ([V | 1] stationary, 65-col output whose
row 64 is the softmax denominator) and is software-pipelined one q-block
behind QK so the PE never stalls on the elementwise pipeline. The Wo
contraction for q-span qb is emitted inside group 1's attention loop as
soon as both groups' outT spans are ready, hiding the output-projection
tail. Reciprocal via DRAM-bounce repack as before."""
import sys

if "/opt/trn_rl_repo" not in sys.path:
    sys.path.insert(0, "/opt/trn_rl_repo")

from contextlib import ExitStack

import ml_dtypes
import numpy as np

B, S, E = 2, 2048, 1024
H = 16
HD = 64
KD = 64
VD = 64
SCALE = 1.0 / np.float32(np.sqrt(np.float32(KD)))
N_CORES = 8
HPC = H // 4  # heads per core = 4
QB = 512  # q-block width
NKC = S // 128  # 16 k-chunks
A16 = np.float64(128.0 / np.log(2.0))  # Schraudolph bf16 scale
B16 = np.float64(127.0 * 128.0 - 7.4)  # Schraudolph bf16 offset

# plan per (qb, kc4) cell; kc4 = kc2//2 indexes [128, 2048] spans.
# 'A' true exp; 'B' DVE fast-exp; 'G' GpSimd fast-exp; 'C' GpSimd mul.
PLAN = {
    (0, 0): "A", (0, 1): "A", (0, 2): "A", (0, 3): "A",
    (1, 0): "A", (1, 1): "C", (1, 2): "A", (1, 3): "A",
    (2, 0): "A", (2, 1): "A", (2, 2): "A", (2, 3): "A",
    (3, 0): "A", (3, 1): "C", (3, 2): "A", (3, 3): "A",
}

_RUNTIME = {}


def _build_nc(repeat=1):
    import concourse.bass as bass
    import concourse.tile as tile
    from concourse import mybir, bacc

    F32 = mybir.dt.float32
    F32R = mybir.dt.float32r
    BF16 = mybir.dt.bfloat16
    I16 = mybir.dt.int16
    Copy = mybir.ActivationFunctionType.Copy
    Ident = mybir.ActivationFunctionType.Identity
    Exp = mybir.ActivationFunctionType.Exp
    Add = mybir.AluOpType.add

    nc = bacc.Bacc("TRN2")
    mask_d = nc.dram_tensor("maskt", (128, NKC * S), BF16, kind="ExternalInput")
    wp_d = nc.dram_tensor("wpack", (128, 3 * 64 + 2), F32, kind="ExternalInput")
    wqkb_d = nc.dram_tensor("wqkb", (128, 192), BF16, kind="ExternalInput")
    xtb_d = nc.dram_tensor("xtb", (4 * HD, S), BF16, kind="ExternalInput")
    wo_d = nc.dram_tensor("wo", (4 * VD, E), BF16, kind="ExternalInput")
    out_d = nc.dram_tensor("partial", (S, E), BF16, kind="ExternalOutput")

    with tile.TileContext(nc) as tc:
        with ExitStack() as ctx:
            const = ctx.enter_context(tc.tile_pool(name="const", bufs=1))
            qkp = ctx.enter_context(tc.tile_pool(name="qkp", bufs=2))
            vtp = ctx.enter_context(tc.tile_pool(name="vtp", bufs=2))
            ptp = ctx.enter_context(tc.tile_pool(name="ptp", bufs=2))
            ctp = ctx.enter_context(tc.tile_pool(name="ctp", bufs=2))
            otbp = ctx.enter_context(tc.tile_pool(name="otbp", bufs=2))
            ot2p = ctx.enter_context(tc.tile_pool(name="ot2p", bufs=2))
            denp = ctx.enter_context(tc.tile_pool(name="denp", bufs=2))
            wst = ctx.enter_context(tc.tile_pool(name="wst", bufs=2))
            drp = ctx.enter_context(tc.tile_pool(name="drp", bufs=2, space="DRAM"))
            simp = ctx.enter_context(tc.tile_pool(name="simp", bufs=6, space="PSUM"))
            accp = ctx.enter_context(tc.tile_pool(name="accp", bufs=2, space="PSUM"))

            # ---- constant loads ----
            wp_sb = const.tile([128, 3 * 64 + 2], F32, tag="wp")
            nc.sync.dma_start(out=wp_sb.bitcast(F32R), in_=wp_d[:, :].bitcast(F32R))
            wq_sb = wp_sb[:, 0:64]
            wk_sb = wp_sb[:, 64:128]
            wv_sb = wp_sb[:, 128:192]
            bq_sb = wp_sb[:, 192:193]
            bk_sb = wp_sb[:, 193:194]
            mask_sb = const.tile([128, NKC * S], BF16, tag="mask")
            for kc in range(NKC):
                eng = nc.scalar if kc % 2 == 0 else nc.sync
                eng.dma_start(
                    out=mask_sb[:, kc * S : (kc + 1) * S],
                    in_=mask_d[:, kc * S : (kc + 1) * S],
                )
            wqkb_sb = const.tile([128, 192], BF16, tag="wqkb")
            nc.sync.dma_start(out=wqkb_sb, in_=wqkb_d[:, :])
            wo_sb = []
            for g in range(2):
                t = const.tile([128, E], BF16, tag=f"wo{g}")
                nc.scalar.dma_start(out=t, in_=wo_d[g * 128 : (g + 1) * 128, :])
                wo_sb.append(t)

            def emit_wo(qc):
                ost = wst.tile([128, E], BF16, tag="wst", name=f"ost_{qc}")
                for e2 in range(2):
                    wo_ps = simp.tile([128, 512], F32, tag="sim", name=f"wops_{qc}_{e2}")
                    for gi in range(2):
                        nc.tensor.matmul(
                            wo_ps[:, :],
                            ot2s[gi][:, qc * 128 : (qc + 1) * 128],
                            wo_sb[gi][:, e2 * 512 : (e2 + 1) * 512],
                            start=(gi == 0),
                            stop=(gi == 1),
                        )
                    dst = ost[:, e2 * 512 : (e2 + 1) * 512]
                    if (qc + e2) % 2 == 0:
                        nc.scalar.activation(dst, wo_ps[:, :], Copy)
                    else:
                        nc.vector.tensor_copy(dst, wo_ps[:, :])
                eng = nc.sync if qc % 2 == 0 else nc.scalar
                eng.dma_start(out=out_d[qc * 128 : (qc + 1) * 128, :], in_=ost)

            for rep in range(repeat):
                ot2s = []
                # pipeline state carried across group boundaries
                prev = None         # dict(pts, vts, ot2, qb) awaiting PV
                out_pending = None  # (recbs, pvs, qb, g, ot2) awaiting out-mul
                wo_pending = []     # qc indices ready for output projection

                def emit_chain(pvs):
                    """Start denominator -> reciprocal -> broadcast chain."""
                    recbs = []
                    for p01 in range(2):
                        den = denp.tile([1, QB], F32, tag="den")
                        nc.scalar.activation(den, pvs[p01][64:65, :], Copy)
                        dden = drp.tile([1, QB], F32, tag="dden")
                        nc.sync.dma_start(out=dden, in_=den)
                        dpk = denp.tile([128, QB // 128], F32, tag="dpk")
                        nc.sync.dma_start(
                            out=dpk,
                            in_=dden.rearrange("a (p f) -> (a p) f", p=128),
                        )
                        rpk = denp.tile([128, QB // 128], BF16, tag="rpk")
                        with nc.allow_low_precision(reason="bf16 1/den: ~0.2% softmax scale error, within tolerance"):
                            nc.vector.reciprocal(rpk, dpk)
                        drec = drp.tile([1, QB], BF16, tag="drec")
                        nc.sync.dma_start(
                            out=drec.rearrange("a (p f) -> (a p) f", p=128),
                            in_=rpk,
                        )
                        recb = denp.tile([64, QB], BF16, tag="recb")
                        nc.sync.dma_start(
                            out=recb,
                            in_=bass.AP(
                                tensor=drec.tensor,
                                offset=drec.offset,
                                ap=[[0, 64]] + [list(a) for a in drec.ap[1:]],
                            ),
                        )
                        recbs.append(recb)
                    return recbs

                def emit_outmul(recbs, pvs, fqb, fg, fot2):
                    fqsl = slice(fqb * QB, (fqb + 1) * QB)
                    # pair A: lane-aligned direct write into ot2 rows 0-63
                    nc.vector.tensor_mul(
                        fot2[0:64, fqsl], pvs[0][0:64, :], recbs[0]
                    )
                    # pair B: psum rows 0-63 -> ot2 rows 64-127 via DMA shift
                    otb = otbp.tile([64, QB], BF16, tag="otb")
                    nc.vector.tensor_mul(otb, pvs[1][0:64, :], recbs[1])
                    nc.scalar.dma_start(out=fot2[64:128, fqsl], in_=otb)

                for g in range(2):
                    xtb_g = const.tile([128, S], BF16, tag="xtb", name=f"xtb_g{g}_r{rep}")
                    eng = nc.sync if g == 0 else nc.scalar
                    eng.dma_start(
                        out=xtb_g, in_=xtb_d[g * 128 : (g + 1) * 128, :]
                    )
                    # ---- Q/K projections: row+col tiled (T0 / T10) ----
                    qt = qkp.tile([128, S], BF16, tag="qt", name=f"qt_g{g}_r{rep}")
                    kt = qkp.tile([128, S], BF16, tag="kt", name=f"kt_g{g}_r{rep}")
                    for sp in range(S // 512):
                        ssl = slice(sp * 512, (sp + 1) * 512)
                        for dst, wcol, b_sb in ((qt, 0, bq_sb), (kt, 64, bk_sb)):
                            ps = simp.tile([128, 512], F32, tag="sim")
                            nc.tensor.matmul(
                                ps[0:64, :],
                                wqkb_sb[0:64, wcol : wcol + 64],
                                xtb_g[0:64, ssl],
                                start=True, stop=True,
                                tile_position=(0, 0),
                            )
                            nc.tensor.matmul(
                                ps[64:128, :],
                                wqkb_sb[64:128, wcol : wcol + 64],
                                xtb_g[64:128, ssl],
                                start=True, stop=True,
                                tile_position=(64, 64),
                            )
                            nc.scalar.activation(
                                dst[:, ssl], ps[:, :], Ident, bias=b_sb[0:128, :]
                            )

                    # V tiles ([V | 1] layout); projections interleave into qb 0
                    vts = []
                    for p01 in range(2):
                        vt = vtp.tile(
                            [128, NKC * 65], BF16, tag=f"vt{p01}",
                            name=f"v_g{g}_{p01}_r{rep}",
                        )
                        ones_ap = vt.rearrange("p (c k) -> p c k", k=65)[:, :, 64:65]
                        nc.gpsimd.memset(ones_ap, 1.0)
                        vts.append(vt)

                    ot2 = ot2p.tile([128, S], BF16, tag="ot2", name=f"ot2_g{g}_r{rep}")
                    ot2s.append(ot2)

                    for qb in range(S // QB):
                        qsl = slice(qb * QB, (qb + 1) * QB)
                        pts = [
                            ptp.tile([128, NKC * QB], BF16, tag=f"pt{p}",
                                     name=f"pt{p}_g{g}_q{qb}_r{rep}")
                            for p in range(2)
                        ]
                        if prev is not None:
                            pvs_prev = []
                        ctmps = [None, None]
                        vps = None
                        for kc in range(NKC):
                            kc2 = kc // 2
                            j2 = kc % 2
                            plan = PLAN[(qb, kc2 // 2)]
                            # PV for the previous q-block in 8-matmul bursts
                            # (kc 4/8/12 + tail): operands are always ready and
                            # the dense burst keeps the PE activity monitor hot.
                            # pvs allocation at kc4 lands just after the kc3
                            # out-mul frees the previous accumulators (acc=2).
                            if prev is not None and kc in (4, 8, 12):
                                if kc == 4:
                                    pvs_prev.extend(
                                        accp.tile([65, QB], F32, tag="acc",
                                                  name=f"pv{p}_g{g}_q{qb}m1_r{rep}")
                                        for p in range(2)
                                    )
                                for c in range(kc - 4, kc):
                                    for p01 in range(2):
                                        nc.tensor.matmul(
                                            pvs_prev[p01][:, :],
                                            prev["vts"][p01][:, c * 65 : (c + 1) * 65],
                                            prev["pts"][p01][:, c * QB : (c + 1) * QB],
                                            start=(c == 0),
                                            stop=False,
                                        )
                            sims = [simp.tile([128, 512], F32, tag="sim", name=f"sim{p}_g{g}_q{qb}_k{kc}_r{rep}") for p in range(2)]
                            for p01 in range(2):
                                rsl = slice(p01 * 64, p01 * 64 + 64)
                                nc.tensor.matmul(
                                    sims[p01][:, :],
                                    kt[rsl, kc * 128 : (kc + 1) * 128],
                                    qt[rsl, qsl],
                                    start=True, stop=True,
                                    tile_position=(p01 * 64, 0),
                                )
                            if qb == 0 and kc % 4 == 0:
                                # V projection burst: chunks kc..kc+3, row-tiled
                                vc4 = kc // 4
                                vps = [simp.tile([128, 256], F32, tag="sim",
                                                 name=f"vps{p}_g{g}_c{vc4}_r{rep}")
                                       for p in range(2)]
                                for j in range(4):
                                    sc = kc + j
                                    for p01 in range(2):
                                        rsl = slice(p01 * 64, p01 * 64 + 64)
                                        nc.tensor.matmul(
                                            vps[p01][:, j * 64 : (j + 1) * 64],
                                            xtb_g[rsl, sc * 128 : (sc + 1) * 128],
                                            wqkb_sb[rsl, 128:192],
                                            start=True, stop=True,
                                            tile_position=(p01 * 64, 0),
                                        )
                                for p01 in range(2):
                                    dst = vts[p01].rearrange("p (c k) -> p c k", k=65)[
                                        :, vc4 * 4 : (vc4 + 1) * 4, 0:64
                                    ]
                                    vsrc = vps[p01].rearrange("p (c k) -> p c k", k=64)
                                    if p01 == 0:
                                        nc.scalar.activation(dst, vsrc, Copy)
                                    else:
                                        nc.vector.tensor_copy(dst, vsrc)
                            moff = (kc2 * (S // QB) + qb) * 1024 + j2 * 512
                            span = slice(kc * QB, (kc + 1) * QB)
                            for p01 in range(2):
                                if plan == "C":
                                    if j2 == 0:
                                        ctmps[p01] = ctp.tile(
                                            [128, 1024], BF16, tag=f"ct{p01}",
                                            name=f"ct{p01}_g{g}_q{qb}_k{kc}_r{rep}",
                                        )
                                    nc.scalar.activation(
                                        ctmps[p01][:, j2 * 512 : (j2 + 1) * 512],
                                        sims[p01][:, :], Copy,
                                    )
                                    if j2 == 1:
                                        moff2 = (kc2 * (S // QB) + qb) * 1024
                                        span2k = slice((kc - 1) * QB, (kc + 1) * QB)
                                        nc.gpsimd.tensor_mul(
                                            pts[p01][:, span2k], ctmps[p01],
                                            mask_sb[:, moff2 : moff2 + 1024],
                                        )
                                else:
                                    nc.vector.tensor_mul(
                                        pts[p01][:, span], sims[p01][:, :],
                                        mask_sb[:, moff : moff + 512],
                                    )
                            if kc % 4 == 3:
                                span2 = slice((kc - 3) * QB, (kc + 1) * QB)
                                for p01 in range(2):
                                    seg = pts[p01][:, span2]
                                    if plan == "A":
                                        nc.scalar.activation(seg, seg, Exp)
                                    else:  # B, C -> DVE fast-exp add
                                        nc.vector.tensor_scalar(
                                            seg.bitcast(I16), seg, float(B16), None,
                                            op0=Add,
                                        )
                            if kc == 3 and out_pending is not None:
                                wo_g, wo_qb = out_pending[3], out_pending[2]
                                emit_outmul(*out_pending)
                                out_pending = None
                                if wo_g == 1:
                                    wo_pending = list(range(4 * wo_qb, 4 * wo_qb + 4))
                            if kc in (6, 8, 10, 12) and wo_pending:
                                emit_wo(wo_pending.pop(0))
                        while wo_pending:
                            emit_wo(wo_pending.pop(0))
                        if prev is not None:
                            for c in range(12, NKC):
                                for p01 in range(2):
                                    nc.tensor.matmul(
                                        pvs_prev[p01][:, :],
                                        prev["vts"][p01][:, c * 65 : (c + 1) * 65],
                                        prev["pts"][p01][:, c * QB : (c + 1) * QB],
                                        start=False,
                                        stop=(c == NKC - 1),
                                    )
                            out_pending = (emit_chain(pvs_prev), pvs_prev,
                                           prev["qb"], prev["g"], prev["ot2"])
                        prev = {"pts": pts, "vts": vts, "ot2": ot2, "qb": qb, "g": g}

                # ---- tail: PV + finish for (g=1, qb=3) ----
                if out_pending is not None:
                    emit_outmul(*out_pending)
                    wo_pending = list(range(8, 12))
                    out_pending = None
                pvs_last = [
                    accp.tile([65, QB], F32, tag="acc", name=f"pv{p}_last_r{rep}")
                    for p in range(2)
                ]
                for c in range(NKC):
                    for p01 in range(2):
                        nc.tensor.matmul(
                            pvs_last[p01][:, :],
                            prev["vts"][p01][:, c * 65 : (c + 1) * 65],
                            prev["pts"][p01][:, c * QB : (c + 1) * QB],
                            start=(c == 0),
                            stop=(c == NKC - 1),
                        )
                ch_last = emit_chain(pvs_last)
                while wo_pending:
                    emit_wo(wo_pending.pop(0))
                emit_outmul(ch_last, pvs_last, 3, 1, ot2s[1])

                # ---- output projection (remaining spans) ----
                for qc in range(12, 16):
                    emit_wo(qc)
    nc.finalize()
    return nc


def _build_runner(repeat=1):
    """Compile once. Returns an object with prep/exec/reduce/run (see use
    in kernel() and test.py)."""
    import jax
    import jax.numpy as jnp
    import numpy as _np
    from jax.experimental.shard_map import shard_map
    from jax.sharding import Mesh, NamedSharding, PartitionSpec

    from concourse import mybir
    from concourse.bass2jax import (
        _bass_exec_p,
        install_neuronx_cc_hook,
        partition_id_tensor,
    )

    nc = _build_nc(repeat=repeat)
    install_neuronx_cc_hook()
    partition_name = nc.partition_id_tensor.name if nc.partition_id_tensor else None

    replicated = {"maskt", "wpack", "wqkb"}

    in_names, out_names, out_avals, out_shapes, out_dtypes = [], [], [], [], []
    for alloc in nc.m.functions[0].allocations:
        if not isinstance(alloc, mybir.MemoryLocationSet):
            continue
        name = alloc.memorylocations[0].name
        if alloc.kind == "ExternalInput":
            if name != partition_name:
                in_names.append(name)
        elif alloc.kind == "ExternalOutput":
            out_names.append(name)
            shape = tuple(alloc.tensor_shape)
            dtype = mybir.dt.np(alloc.dtype)
            out_avals.append(jax.core.ShapedArray(shape, dtype))
            out_shapes.append(shape)
            out_dtypes.append(dtype)

    n_params = len(in_names)
    n_outs = len(out_names)
    all_in_names = list(in_names) + list(out_names)
    if partition_name is not None:
        all_in_names.append(partition_name)
    donate = tuple(range(n_params, n_params + n_outs))

    def _body(*args):
        operands = list(args)
        if partition_name is not None:
            operands.append(partition_id_tensor())
        outs = _bass_exec_p.bind(
            *operands,
            out_avals=tuple(out_avals),
            in_names=tuple(all_in_names),
            out_names=tuple(out_names),
            lowering_input_output_aliases=(),
            sim_require_finite=True,
            sim_require_nnan=True,
            nc=nc,
        )
        return tuple(outs)

    devices = jax.devices()[:N_CORES]
    mesh = Mesh(_np.asarray(devices), ("core",))
    shard0 = NamedSharding(mesh, PartitionSpec("core"))
    srepl = NamedSharding(mesh, PartitionSpec())
    in_specs = tuple(
        PartitionSpec() if name in replicated else PartitionSpec("core")
        for name in in_names
    ) + (PartitionSpec("core"),) * n_outs
    out_specs = (PartitionSpec("core"),) * n_outs

    sharded = jax.jit(
        shard_map(
            _body, mesh=mesh, in_specs=in_specs, out_specs=out_specs,
            check_rep=False,
        ),
        donate_argnums=donate,
        keep_unused=True,
    )

    _zeros = jax.jit(
        lambda: tuple(
            jnp.zeros((N_CORES * s[0], *s[1:]), d)
            for s, d in zip(out_shapes, out_dtypes)
        ),
        out_shardings=(shard0,) * n_outs,
    )

    _reduce = jax.jit(
        lambda p: p.reshape(B, 4, S, E).sum(axis=1).reshape(B * S, E),
        out_shardings=shard0,
    )

    def prep(in_maps):
        args = []
        for name in in_names:
            if name in replicated:
                arr = _np.asarray(in_maps[0][name])
                args.append(jax.device_put(arr, srepl))
            else:
                arr = _np.concatenate(
                    [_np.asarray(m[name]) for m in in_maps], axis=0
                )
                args.append(jax.device_put(arr, shard0))
        return args

    def make_zeros():
        return _zeros()

    def exec_device(args, zeros=None):
        if zeros is None:
            zeros = _zeros()
        outs = sharded(*args, *zeros)
        return jax.block_until_ready(outs[0])

    def exec_async(args, zeros):
        return sharded(*args, *zeros)[0]

    def reduce_device(partials):
        return jax.block_until_ready(_reduce(partials))

    def run(in_maps):
        partials = exec_device(prep(in_maps))
        return _np.asarray(reduce_device(partials))  # (B*S, E)

    class R:
        pass

    r = R()
    r.nc = nc
    r.prep = prep
    r.make_zeros = make_zeros
    r.exec_device = exec_device
    r.exec_async = exec_async
    r.reduce_device = reduce_device
    r.run = run
    return r


def _runtime(repeat=1):
    if repeat not in _RUNTIME:
        _RUNTIME[repeat] = _build_runner(repeat=repeat)
    return _RUNTIME[repeat]


def make_in_maps(x, mask, Wq, bq, Wk, bk, Wv, bv, Wo, bo):
    bf16 = ml_dtypes.bfloat16
    x = np.asarray(x, np.float32)
    m = np.asarray(mask, np.float32).T  # [k, q]
    # device layout: [128, (kc2, qb, j, ql)]; each mul reads one flat
    # [128, 1024] span at moff=(kc2*4+qb)*1024
    maskT = np.ascontiguousarray(
        m.reshape(NKC // 2, 2, 128, S // QB, QB)
        .transpose(2, 0, 3, 1, 4)
        .reshape(128, NKC * S)
    ).astype(np.float32)
    # scale fast-exp regions by A16 (Schraudolph): all plans except 'A'
    mview = maskT.reshape(128, NKC // 2, S // QB, 2 * QB)
    for kc2 in range(NKC // 2):
        for qb in range(S // QB):
            if PLAN[(qb, kc2 // 2)] != "A":
                mview[:, kc2, qb, :] *= np.float32(A16)
    maskT = maskT.astype(bf16)

    wq_s = (np.asarray(Wq, np.float32) * SCALE).astype(np.float32)
    bq_s = (np.asarray(bq, np.float32) * SCALE).astype(np.float32)
    wq2 = np.concatenate([wq_s, wq_s], axis=0)
    wk2 = np.concatenate([np.asarray(Wk, np.float32)] * 2, axis=0)
    wv2 = np.concatenate([np.asarray(Wv, np.float32)] * 2, axis=0)
    bq2 = np.concatenate([bq_s, bq_s])[:, None].astype(np.float32)
    bk2 = np.concatenate([np.asarray(bk, np.float32)] * 2)[:, None].astype(np.float32)
    wpack = np.ascontiguousarray(
        np.concatenate([wq2, wk2, wv2, bq2, bk2], axis=1), np.float32
    )
    wqkb = np.ascontiguousarray(
        np.concatenate([wq2, wk2, wv2], axis=1)
    ).astype(bf16)

    in_maps = []
    for c in range(N_CORES):
        b = c // 4
        h0 = (c % 4) * HPC
        r0 = h0 * HD
        xtb = np.ascontiguousarray(x[b].T[r0 : r0 + HPC * HD, :]).astype(bf16)
        wo = np.ascontiguousarray(np.asarray(Wo, np.float32)[r0 : r0 + HPC * VD, :]).astype(bf16)
        in_maps.append(
            {
                "xtb": xtb,
                "maskt": maskT,
                "wpack": wpack,
                "wqkb": wqkb,
                "wo": wo,
            }
        )
    return in_maps


def kernel(x, mask, Wq, bq, Wk, bk, Wv, bv, Wo, bo):
    r = _runtime()
    in_maps = make_in_maps(x, mask, Wq, bq, Wk, bk, Wv, bv, Wo, bo)
    flat = r.run(in_maps)  # (B*S, E), per-batch partials already summed
    Wo32 = np.asarray(Wo, np.float32)
    crow = np.asarray(bo, np.float32) + np.tile(np.asarray(bv, np.float32), H) @ Wo32
    out = flat.reshape(B, S, E) + crow[None, None, :]
    return out.astype(np.float32)
